# revision 1
# baseline (speedup 1.0000x reference)
"""Trainium2 Bass kernel for an attention-LSTM caption decoder.

Math notes (verified against the reference on CPU):
  - num_pixels == 1 makes the softmax attention exactly a no-op: alpha == 1.0,
    so awe = sigmoid(h @ W_beta) * features. W_enc/W_dec/W_full are unused.
  - Masked (b, t) rows (t >= len[b]) never re-activate and never influence
    active rows, so h/c freezing can be dropped; only output masking matters.
    We compute/emit only the active rows (lengths are sorted descending, so
    the active rows at step t are a prefix of the batch).

Distribution (8 cores): recurrence replicated on every core; fc weight,
fc bias and the output vocab dim sharded 8-way (tensor parallel). The
embedding table is fully resident in each core's HBM and gathered on-device.

Schedule: the emb contribution to the LSTM gates is precomputed for all
timesteps as E = emb @ W_ih[:, :H].T + b (full-width matmuls, staged via
DRAM), and the vocab projection is interleaved into each step's
pointwise-chain gap to keep the PE dense and HAM-warm. All matmul operands
are bf16 (products exact on PE, fp32 PSUM accumulation; cell state c and
all pointwise math stay fp32) - verified 3.6e-3 end-to-end on CPU.
"""

import numpy as np

from concourse import bacc, bass, library_config, mybir, tile
from concourse.bass_utils import run_bass_kernel_spmd

F32 = mybir.dt.float32
BF16 = mybir.dt.bfloat16
I16 = mybir.dt.int16

B = 64
H = 512
T = 20
V = 50257
NCORES = 8
VS = 6284            # per-core vocab shard (8 * 6284 = 50272 >= 50257)
VPAD = NCORES * VS
KC_H = H // 128      # 4 k-chunks per 512-wide contraction
GATE_N = 4 * H       # 2048
HALF = 32768         # embedding table split point (int16 index range)

# emb row order in embT/E: gather rows j=0..1215 are (t=1..19, b); the
# features block (t=0 input) lives at rows 1216..1279.
N_GATHER = (T - 1) * B           # 1216
EMB_ROWS = T * B                 # 1280
FEAT_OFF = N_GATHER              # 1216
GATHER_GROUPS = [(0, 384), (384, 768), (768, 1152), (1152, 1216)]
VCHUNKS = [(i * 512, min(512, VS - i * 512)) for i in range(13)]


def _pack_k(w):
    """[K, N] -> [128, K//128, N] with the contraction dim on partitions."""
    k, n = w.shape
    assert k % 128 == 0
    return np.ascontiguousarray(w.reshape(k // 128, 128, n).transpose(1, 0, 2))


def _pack_idx(a):
    """(n,) int16 -> [128, n//16]; j = s*16 + p wrapping, and the block must
    be replicated into each of the 8 GPSIMD Q7 cores' 16-partition groups
    (the simulator only reads partitions 0-15, but hardware Q7 core c reads
    partitions 16c..16c+15)."""
    n = a.shape[0]
    assert n % 16 == 0
    out = np.zeros((128, n // 16), np.int16)
    for c in range(8):
        out[16 * c : 16 * c + 16, :] = a.reshape(n // 16, 16).T
    return out


def _host_prep(inputs):
    import ml_dtypes

    bf16 = ml_dtypes.bfloat16
    f32 = np.float32
    feats = np.asarray(inputs["features"], f32)
    caps = np.asarray(inputs["captions"]).astype(np.int64)
    lens = np.asarray(inputs["lengths"]).reshape(-1).astype(np.int64)
    table = np.asarray(inputs["embed_table"], f32)

    W_ih = np.asarray(inputs["W_ih"], f32)
    W_hh = np.asarray(inputs["W_hh"], f32)
    b_ih = np.asarray(inputs["b_ih"], f32)
    b_hh = np.asarray(inputs["b_hh"], f32)

    # ragged-batch packing (lengths sorted descending by construction)
    b_t = [int((lens > t).sum()) for t in range(T)]
    off = np.concatenate([[0], np.cumsum(b_t)]).astype(np.int64)
    p_total = int(off[-1])
    p_pad = ((p_total + 127) // 128) * 128

    # embedding gather indices, t-major (t=1..19)
    idx_flat = caps.T.reshape(-1)  # j = (t-1)*64 + b
    is_hi = idx_flat >= HALF
    idx_lo = np.where(is_hi, 0, idx_flat).astype(np.int16)
    idx_hi = np.where(is_hi, idx_flat - HALF, 0).astype(np.int16)
    sel = np.zeros((128, EMB_ROWS // 128), f32)
    for j in range(N_GATHER):
        sel[j % 128, j // 128] = 1.0 if is_hi[j] else 0.0

    w2emb = W_ih.T[:H]            # [512, 2048] emb input rows
    w2ah = np.vstack([W_ih.T[H:], W_hh.T])  # [1024, 2048] awe+h input rows

    common = {
        "features": feats,
        "table": table,
        "w2e": _pack_k(w2emb).astype(bf16),
        "w2ah": _pack_k(w2ah).astype(bf16),
        "b2rep": np.ascontiguousarray(
            np.tile((b_ih + b_hh)[None, :], (128, 1)).astype(f32)
        ),
        "wbeta": _pack_k(np.asarray(inputs["W_beta"], f32)).astype(bf16),
        "bbetarow": np.asarray(inputs["b_beta"], f32).reshape(1, H).astype(bf16),
        "whinit": _pack_k(np.asarray(inputs["W_hinit"], f32)).astype(bf16),
        "bhinitrow": np.asarray(inputs["b_hinit"], f32).reshape(1, H).astype(bf16),
        "wcinit": _pack_k(np.asarray(inputs["W_cinit"], f32)).astype(bf16),
        "bcinitrow": np.asarray(inputs["b_cinit"], f32).reshape(1, H).astype(bf16),
        "ident": np.eye(128, dtype=f32).astype(bf16),
        "ones": np.ones((1, B), bf16),
        "sel": sel,
    }
    for g, (a, b) in enumerate(GATHER_GROUPS):
        common[f"idxlo{g}"] = _pack_idx(idx_lo[a:b])
        common[f"idxhi{g}"] = _pack_idx(idx_hi[a:b])

    W_fc = np.asarray(inputs["W_fc"], f32)
    b_fc = np.asarray(inputs["b_fc"], f32)
    wfc_pad = np.zeros((H, VPAD), f32)
    wfc_pad[:, :V] = W_fc
    bfc_pad = np.zeros(VPAD, f32)
    bfc_pad[:V] = b_fc

    in_maps = []
    for k in range(NCORES):
        m = dict(common)
        m["wfc"] = _pack_k(wfc_pad[:, k * VS : (k + 1) * VS]).astype(bf16)
        m["bfcrep"] = np.ascontiguousarray(
            np.tile(bfc_pad[k * VS : (k + 1) * VS][None, :], (128, 1))
        ).astype(bf16)
        in_maps.append(m)

    meta = {"b_t": b_t, "off": off, "p_total": p_total, "p_pad": p_pad}
    return in_maps, meta


def build_program(meta):
    """Build the (SPMD-identical) Bass program. Per-core differences are data
    only (wfc / bfcrep shards)."""
    b_t = meta["b_t"]
    off = [int(x) for x in meta["off"]]
    p_total = meta["p_total"]
    p_pad = meta["p_pad"]
    mv = p_pad // 128

    nc = bacc.Bacc(num_swdge_queues=2)

    feats_d = nc.declare_dram_parameter("features", [B, H], F32, isOutput=False)
    table_d = nc.declare_dram_parameter("table", [V, H], F32, isOutput=False)
    w2e_d = nc.declare_dram_parameter("w2e", [128, KC_H, GATE_N], BF16, isOutput=False)
    w2ah_d = nc.declare_dram_parameter("w2ah", [128, 8, GATE_N], BF16, isOutput=False)
    b2rep_d = nc.declare_dram_parameter("b2rep", [128, GATE_N], F32, isOutput=False)
    wbeta_d = nc.declare_dram_parameter("wbeta", [128, KC_H, H], BF16, isOutput=False)
    bbetarow_d = nc.declare_dram_parameter("bbetarow", [1, H], BF16, isOutput=False)
    whinit_d = nc.declare_dram_parameter("whinit", [128, KC_H, H], BF16, isOutput=False)
    bhinitrow_d = nc.declare_dram_parameter("bhinitrow", [1, H], BF16, isOutput=False)
    wcinit_d = nc.declare_dram_parameter("wcinit", [128, KC_H, H], BF16, isOutput=False)
    bcinitrow_d = nc.declare_dram_parameter("bcinitrow", [1, H], BF16, isOutput=False)
    ident_d = nc.declare_dram_parameter("ident", [128, 128], BF16, isOutput=False)
    ones_d = nc.declare_dram_parameter("ones", [1, B], BF16, isOutput=False)
    sel_d = nc.declare_dram_parameter("sel", [128, EMB_ROWS // 128], F32, isOutput=False)
    idx_d = {}
    for g, (a, b) in enumerate(GATHER_GROUPS):
        w = (b - a) // 16
        idx_d[("lo", g)] = nc.declare_dram_parameter(f"idxlo{g}", [128, w], I16, isOutput=False)
        idx_d[("hi", g)] = nc.declare_dram_parameter(f"idxhi{g}", [128, w], I16, isOutput=False)
    wfc_d = nc.declare_dram_parameter("wfc", [128, KC_H, VS], BF16, isOutput=False)
    bfcrep_d = nc.declare_dram_parameter("bfcrep", [128, VS], BF16, isOutput=False)
    out_d = nc.declare_dram_parameter("out", [p_pad, VS], F32, isOutput=True)

    # E = emb @ W_ih_emb.T + (b_ih + b_hh), staged via DRAM. Row m at
    # [m % 128, m // 128, :].
    e_d = nc.dram_tensor("e", [128, EMB_ROWS // 128, GATE_N], BF16)

    def mm(out, lhsT, rhs, start, stop):
        nc.tensor.matmul(out, lhsT, rhs, start=start, stop=stop)

    # vocab work items; hall columns for mc are complete after step
    # ready[mc]'s h-transposes
    mc_ready = []
    for mc in range(mv):
        need = (mc + 1) * 128
        r = T - 1
        for t in range(T):
            if off[t + 1] >= need:
                r = t
                break
        mc_ready.append(r)
    vqueue = [(mc, vo, vw) for mc in range(mv) for (vo, vw) in VCHUNKS]

    # per-step hall write segments: (mc, local_lo, src_lo, n)
    hall_segs = []
    for t in range(T):
        segs = []
        lo, n = off[t], b_t[t]
        while n > 0:
            mc = lo // 128
            ll = lo % 128
            take = min(128 - ll, n)
            segs.append((mc, ll, lo - off[t], take))
            lo += take
            n -= take
        hall_segs.append(segs)

    with tile.TileContext(nc) as tc:
        nc.gpsimd.load_library(library_config.mlp)
        with (
            tc.tile_pool(name="const", bufs=1) as constp,
            tc.tile_pool(name="res", bufs=1) as resp,
            tc.tile_pool(name="state", bufs=1) as statep,
        ):
            ident = constp.tile([128, 128], BF16)
            nc.sync.dma_start(ident[:], ident_d[:])
            ones = constp.tile([1, B], BF16)
            nc.sync.dma_start(ones[:], ones_d[:])
            feats = constp.tile([B, H], F32)
            nc.sync.dma_start(feats[:], feats_d[:])
            bbetarow = constp.tile([1, H], BF16)
            nc.sync.dma_start(bbetarow[:], bbetarow_d[:])
            featT = constp.tile([128, KC_H, B], BF16)

            w2ah = resp.tile([128, 8, GATE_N], BF16)
            nc.scalar.dma_start(w2ah[:], w2ah_d[:])
            wbeta = resp.tile([128, KC_H, H], BF16)
            nc.scalar.dma_start(wbeta[:], wbeta_d[:])
            wfc = resp.tile([128, KC_H, VS], BF16)
            nc.scalar.dma_start(wfc[:], wfc_d[:])
            bfc = resp.tile([128, VS], BF16)
            nc.scalar.dma_start(bfc[:], bfcrep_d[:])
            hall_t = [
                resp.tile([128, KC_H, 128], BF16, tag=f"hall{mc}",
                          name=f"hall{mc}")
                for mc in range(mv)
            ]
            if p_pad > p_total:
                mc = p_total // 128
                nc.vector.memset(hall_t[mc][:, :, p_total % 128 :], 0.0)
                for m2 in range(mc + 1, mv):
                    nc.vector.memset(hall_t[m2][:], 0.0)

            hT = statep.tile([128, KC_H, B], BF16)
            aweT = statep.tile([128, KC_H, B], BF16)
            c_st = statep.tile([B, H], F32)

            # ================= prep phase =================
            with (
                tc.tile_pool(name="prew", bufs=1) as prew,
                tc.tile_pool(name="emb", bufs=1) as embp,
                tc.tile_pool(name="gath", bufs=1) as gp,
                tc.tile_pool(name="gath2", bufs=2) as gp2,
                tc.tile_pool(name="gtmp", bufs=2) as gtmp,
                tc.tile_pool(name="estag", bufs=2) as ep,
                tc.tile_pool(name="gpsum", bufs=2, space="PSUM") as gps,
                tc.tile_pool(name="epsum", bufs=2, space="PSUM") as eps,
            ):
                w2e = prew.tile([128, KC_H, GATE_N], BF16)
                nc.scalar.dma_start(w2e[:], w2e_d[:])
                b2rep = prew.tile([128, GATE_N], F32)
                nc.scalar.dma_start(b2rep[:], b2rep_d[:])
                embT = embp.tile([128, KC_H, EMB_ROWS], BF16)
                selt = gp.tile([128, EMB_ROWS // 128], F32)
                nc.sync.dma_start(selt[:], sel_d[:])

                # features -> featT (cast to bf16 first), also embT tail block
                for kc in range(KC_H):
                    fb = gtmp.tile([B, 128], BF16, tag="fb")
                    nc.vector.tensor_copy(fb[:], feats[:, kc * 128 : (kc + 1) * 128])
                    tp = gps.tile([128, B], BF16, tag="tp")
                    nc.tensor.transpose(tp[:], fb[:], ident[0:B, 0:B])
                    nc.vector.tensor_copy(featT[:, kc, :], tp[:])
                    nc.vector.tensor_copy(embT[:, kc, FEAT_OFF : FEAT_OFF + B], tp[:])

                # h0 / c0 while the first gathers are in flight
                with (
                    tc.tile_pool(name="initp", bufs=1) as ip,
                    tc.tile_pool(name="ipsum", bufs=1, space="PSUM") as ips,
                ):
                    whinit = ip.tile([128, KC_H, H], BF16)
                    nc.scalar.dma_start(whinit[:], whinit_d[:])
                    wcinit = ip.tile([128, KC_H, H], BF16)
                    nc.scalar.dma_start(wcinit[:], wcinit_d[:])
                    bhinitrow = ip.tile([1, H], BF16)
                    nc.sync.dma_start(bhinitrow[:], bhinitrow_d[:])
                    bcinitrow = ip.tile([1, H], BF16)
                    nc.sync.dma_start(bcinitrow[:], bcinitrow_d[:])

                    hps = ips.tile([B, H], F32, tag="hps")
                    for kc in range(KC_H):
                        mm(hps[:], featT[:, kc, :], whinit[:, kc, :],
                           start=(kc == 0), stop=False)
                    mm(hps[:], ones[:], bhinitrow[:], start=False, stop=True)
                    h0 = ip.tile([B, H], BF16)
                    nc.vector.tensor_copy(h0[:], hps[:])
                    for kc in range(KC_H):
                        tp = gps.tile([128, B], BF16, tag="tp")
                        nc.tensor.transpose(
                            tp[:], h0[:, kc * 128 : (kc + 1) * 128], ident[0:B, 0:B]
                        )
                        nc.vector.tensor_copy(hT[:, kc, :], tp[:])

                    cps = ips.tile([B, H], F32, tag="cps")
                    for kc in range(KC_H):
                        mm(cps[:], featT[:, kc, :], wcinit[:, kc, :],
                           start=(kc == 0), stop=False)
                    mm(cps[:], ones[:], bcinitrow[:], start=False, stop=True)
                    nc.vector.tensor_copy(c_st[:], cps[:])

                # gather + blend + transpose + E-precompute, pipelined by group
                def e_pre(mb):
                    lo_m, width = mb * 128, min(128, EMB_ROWS - mb * 128)
                    for nc4 in range(4):
                        pse = eps.tile([128, 512], F32, tag="pse")
                        for kc in range(KC_H):
                            mm(pse[:width, :],
                               embT[:, kc, lo_m : lo_m + width],
                               w2e[:, kc, nc4 * 512 : (nc4 + 1) * 512],
                               start=(kc == 0), stop=(kc == KC_H - 1))
                        est = ep.tile([128, 512], BF16, tag="est")
                        nc.vector.tensor_add(
                            est[:width, :], pse[:width, :],
                            b2rep[:width, nc4 * 512 : (nc4 + 1) * 512],
                        )
                        nc.sync.dma_start(
                            e_d[:width, mb, nc4 * 512 : (nc4 + 1) * 512],
                            est[:width, :],
                        )

                for g, (a, b) in enumerate(GATHER_GROUPS):
                    n = b - a
                    nblk = (n + 127) // 128
                    lo = gp2.tile([128, 3, H], F32, tag="glo")
                    hi = gp2.tile([128, 3, H], F32, tag="ghi")
                    idxlo = gp.tile([128, 24], I16, tag="ilo", bufs=2)
                    idxhi = gp.tile([128, 24], I16, tag="ihi", bufs=2)
                    w = n // 16
                    nc.sync.dma_start(idxlo[:, :w], idx_d[("lo", g)][:])
                    nc.sync.dma_start(idxhi[:, :w], idx_d[("hi", g)][:])
                    nc.gpsimd.dma_gather(
                        lo[:, :nblk, :], table_d[0:HALF, :], idxlo[:, :w],
                        num_idxs=n, num_idxs_reg=n, elem_size=H,
                    )
                    nc.gpsimd.dma_gather(
                        hi[:, :nblk, :], table_d[HALF:V, :], idxhi[:, :w],
                        num_idxs=n, num_idxs_reg=n, elem_size=H, queue_num=1,
                    )
                    for lb in range(nblk):
                        gblk = (a // 128) + lb
                        rows = min(128, n - lb * 128)
                        d = gtmp.tile([128, H], F32, tag="d", bufs=1)
                        tmp = gtmp.tile([128, H], BF16, tag="tmp")
                        nc.vector.tensor_sub(
                            d[:rows, :], hi[:rows, lb, :], lo[:rows, lb, :]
                        )
                        nc.vector.scalar_tensor_tensor(
                            tmp[:rows, :], d[:rows, :],
                            selt[:rows, gblk : gblk + 1], lo[:rows, lb, :],
                            op0=mybir.AluOpType.mult, op1=mybir.AluOpType.add,
                        )
                        for kc in range(KC_H):
                            tp2 = gps.tile([128, 128], BF16, tag="tp2")
                            nc.tensor.transpose(
                                tp2[:, :rows],
                                tmp[:rows, kc * 128 : (kc + 1) * 128],
                                ident[0:rows, 0:rows],
                            )
                            nc.vector.tensor_copy(
                                embT[:, kc, gblk * 128 : gblk * 128 + rows],
                                tp2[:, :rows],
                            )
                    # E rows for this group's blocks (group 3 also covers the
                    # featT tail block)
                    first_mb = a // 128
                    last_mb = (b - 1) // 128 if g < 3 else (EMB_ROWS // 128) - 1
                    for mb in range(first_mb, last_mb + 1):
                        e_pre(mb)

            # ============ recurrence with interleaved vocab ============
            with (
                tc.tile_pool(name="step", bufs=1) as sp,
                tc.tile_pool(name="et", bufs=2) as etp,
                tc.tile_pool(name="stage", bufs=3) as stp,
                tc.tile_pool(name="bps", bufs=1, space="PSUM") as bps_pool,
                tc.tile_pool(name="gatesps", bufs=1, space="PSUM") as gps_pool,
                tc.tile_pool(name="tps", bufs=1, space="PSUM") as tps_pool,
                tc.tile_pool(name="vps", bufs=2, space="PSUM") as vps_pool,
            ):
                def vocab_item(mc, vo, vw):
                    pv = vps_pool.tile([128, 512], F32, tag="pv")
                    for kc in range(KC_H):
                        mm(pv[:, :vw],
                           hall_t[mc][:, kc, :],
                           wfc[:, kc, vo : vo + vw],
                           start=(kc == 0), stop=(kc == KC_H - 1))
                    st = stp.tile([128, 512], F32, tag="st")
                    nc.vector.tensor_add(
                        st[:, :vw], pv[:, :vw], bfc[:, vo : vo + vw]
                    )
                    nc.sync.dma_start(
                        out_d[mc * 128 : (mc + 1) * 128, vo : vo + vw],
                        st[:, :vw],
                    )

                qpos = 0
                for t in range(T):
                    # E_t slice (prefetched; bufs=2 overlaps with compute)
                    m0 = FEAT_OFF if t == 0 else (t - 1) * B
                    p0, gslc = m0 % 128, m0 // 128
                    e_t = etp.tile([B, GATE_N], BF16, tag="e_t")
                    nc.sync.dma_start(e_t[:], e_d[p0 : p0 + B, gslc, :])

                    # beta gate: awe = sigmoid(h @ W_beta + b_beta) * features
                    beta_ps = bps_pool.tile([B, H], F32, tag="beta")
                    for kc in range(KC_H):
                        mm(beta_ps[:], hT[:, kc, :], wbeta[:, kc, :],
                           start=(kc == 0), stop=False)
                    mm(beta_ps[:], ones[:], bbetarow[:], start=False, stop=True)
                    awe = sp.tile([B, H], F32, tag="awe")
                    nc.scalar.activation(
                        awe[:], beta_ps[:], mybir.ActivationFunctionType.Sigmoid
                    )
                    aweb = sp.tile([B, H], BF16, tag="aweb")
                    nc.vector.tensor_mul(aweb[:], awe[:], feats[:])

                    # gates = E_t + [awe; h] @ W2ah  (E first, then h, awe last
                    # so the beta->awe chain overlaps the E/h streams)
                    gates_ps = gps_pool.tile([B, GATE_N], F32, tag="gates")
                    for ncx in range(4):
                        mm(gates_ps[:, ncx * 512 : (ncx + 1) * 512],
                           ident[0:B, 0:B], e_t[:, ncx * 512 : (ncx + 1) * 512],
                           start=True, stop=False)
                    for kc in range(KC_H):
                        for ncx in range(4):
                            mm(gates_ps[:, ncx * 512 : (ncx + 1) * 512],
                               hT[:, kc, :],
                               w2ah[:, 4 + kc, ncx * 512 : (ncx + 1) * 512],
                               start=False, stop=False)
                    for kc in range(KC_H):
                        tp = tps_pool.tile([128, B], BF16, tag="tp")
                        nc.tensor.transpose(
                            tp[:], aweb[:, kc * 128 : (kc + 1) * 128],
                            ident[0:B, 0:B],
                        )
                        nc.vector.tensor_copy(aweT[:, kc, :], tp[:])
                    for kc in range(KC_H):
                        for ncx in range(4):
                            mm(gates_ps[:, ncx * 512 : (ncx + 1) * 512],
                               aweT[:, kc, :],
                               w2ah[:, kc, ncx * 512 : (ncx + 1) * 512],
                               start=False,
                               stop=(kc == KC_H - 1))

                    # vocab quantum: the PE runs these while the pointwise
                    # chain (ACT/DVE) drains; stage-adds are emitted here so
                    # they precede the pointwise ops in the DVE FIFO
                    if t >= 3:
                        navail = sum(
                            1 for (mc, _, _) in vqueue[qpos:] if mc_ready[mc] < t
                        )
                        for _ in range(min(navail, 4)):
                            mc, vo, vw = vqueue[qpos]
                            vocab_item(mc, vo, vw)
                            qpos += 1

                    # LSTM pointwise (PyTorch gate order i, f, g, o)
                    sig_if = sp.tile([B, 2 * H], F32, tag="sig_if")
                    nc.scalar.activation(
                        sig_if[:], gates_ps[:, 0 : 2 * H],
                        mybir.ActivationFunctionType.Sigmoid,
                    )
                    tanh_g = sp.tile([B, H], F32, tag="tanh_g")
                    nc.scalar.activation(
                        tanh_g[:], gates_ps[:, 2 * H : 3 * H],
                        mybir.ActivationFunctionType.Tanh,
                    )
                    sig_o = sp.tile([B, H], F32, tag="sig_o")
                    nc.scalar.activation(
                        sig_o[:], gates_ps[:, 3 * H : 4 * H],
                        mybir.ActivationFunctionType.Sigmoid,
                    )
                    t2 = sp.tile([B, H], F32, tag="t2")
                    nc.vector.tensor_mul(t2[:], sig_if[:, 0:H], tanh_g[:])
                    nc.vector.tensor_mul(c_st[:], c_st[:], sig_if[:, H : 2 * H])
                    nc.vector.tensor_add(c_st[:], c_st[:], t2[:])
                    tanh_c = sp.tile([B, H], F32, tag="tanh_c")
                    nc.scalar.activation(
                        tanh_c[:], c_st[:], mybir.ActivationFunctionType.Tanh
                    )
                    h_new = sp.tile([B, H], BF16, tag="h_new")
                    nc.vector.tensor_mul(h_new[:], sig_o[:], tanh_c[:])

                    for kc in range(KC_H):
                        tp = tps_pool.tile([128, B], BF16, tag="tp")
                        nc.tensor.transpose(
                            tp[:], h_new[:, kc * 128 : (kc + 1) * 128],
                            ident[0:B, 0:B],
                        )
                        nc.vector.tensor_copy(hT[:, kc, :], tp[:])
                        for (mc, ll, sl, seg_n) in hall_segs[t]:
                            nc.vector.tensor_copy(
                                hall_t[mc][:, kc, ll : ll + seg_n],
                                tp[:, sl : sl + seg_n],
                            )

                # drain remaining vocab items
                while qpos < len(vqueue):
                    mc, vo, vw = vqueue[qpos]
                    vocab_item(mc, vo, vw)
                    qpos += 1

    nc.finalize()
    return nc


def kernel(**inputs):
    in_maps, meta = _host_prep(inputs)
    nc = build_program(meta)
    res = run_bass_kernel_spmd(nc, in_maps, core_ids=list(range(NCORES)))
    results = res.results

    b_t = meta["b_t"]
    off = meta["off"]
    full = np.zeros((B, T, VPAD), np.float32)
    for k in range(NCORES):
        o = np.asarray(results[k]["out"])
        for t in range(T):
            bt = b_t[t]
            if bt:
                full[:bt, t, k * VS : (k + 1) * VS] = o[off[t] : off[t] + bt]
    return full[:, :, :V]



# revision 9
# speedup vs baseline: 1.2546x; 1.2546x over previous
"""Trainium2 Bass kernel for an attention-LSTM caption decoder.

Math notes (verified against the reference on CPU):
  - num_pixels == 1 makes the softmax attention exactly a no-op: alpha == 1.0,
    so awe = sigmoid(h @ W_beta) * features. W_enc/W_dec/W_full are unused.
  - Masked (b, t) rows (t >= len[b]) never re-activate and never influence
    active rows, so h/c freezing can be dropped; only output masking matters.
    We compute/emit only the active rows (lengths are sorted descending, so
    the active rows at step t are a prefix of the batch).
  - All biases in setup_inputs() are zero; detected host-side, bias work is
    compiled out (fallback paths are kept for nonzero biases).

Distribution (8 cores): recurrence replicated on every core; fc weight and
the output vocab dim sharded 8-way (tensor parallel). The embedding table
(bf16) is resident in each core's HBM.

Schedule highlights vs the v0 kernel:
  - The embedding gather uses dma_gather(transpose=True) on a bf16 table,
    landing rows directly in the transposed [128, kc, row] layout; a zero
    row appended to each table half turns the lo/hi split into a plain add
    (no select mask). 4 swdge queues, gathers issued before the big weight
    DMAs so they overlap.
  - E = emb @ W_ih_emb.T is kept in SBUF (no DRAM staging) and its blocks
    are computed interleaved with the first recurrence steps.
  - The LSTM gate PSUM is split into four independent 512-wide tiles so the
    pointwise chain on chunk f starts while chunks i/o/g are still being
    accumulated on the PE (awe matmuls are emitted f,i,o,g).
  - Vocab-projection matmuls fill the PE idle window during the pointwise
    chain; their PSUM->SBUF copies alternate between ScalarE and VectorE.
"""

import numpy as np

from concourse import bacc, bass, library_config, mybir, tile
from concourse.bass_utils import run_bass_kernel_spmd

F32 = mybir.dt.float32
BF16 = mybir.dt.bfloat16
I16 = mybir.dt.int16

B = 64
H = 512
T = 20
V = 50257
NCORES = 8
VS = 6284            # per-core vocab shard (8 * 6284 = 50272 >= 50257)
VPAD = NCORES * VS
KC_H = H // 128      # 4 k-chunks per 512-wide contraction
GATE_N = 4 * H       # 2048
SPLIT = 32000        # embedding table split (int16 index range)
NHI = V - SPLIT      # 18257 hi rows
Z_HI = NHI           # local index of the zero row in the hi half

N_GATHER = (T - 1) * B           # 1216 gathered rows, j = (t-1)*64 + b
EMB_BLOCKS = 10                  # E blocks of 128 rows (1280 incl features)
FEAT_OFF = N_GATHER              # features rows live at 1216..1279 (block 9 hi)
GATHER_GROUPS = [(0, 384), (384, 768), (768, 1152), (1152, 1280)]
VCHUNKS = [(i * 512, min(512, VS - i * 512)) for i in range(13)]
VOCAB_CAP = 5                    # max vocab items interleaved per step


def _pack_k(w):
    """[K, N] -> [128, K//128, N] with the contraction dim on partitions."""
    k, n = w.shape
    assert k % 128 == 0
    return np.ascontiguousarray(w.reshape(k // 128, 128, n).transpose(1, 0, 2))


def _pack_idx(a):
    """(n,) int16 -> [128, n//16]; j = s*16 + p wrapping, replicated into each
    of the 8 GPSIMD Q7 cores' 16-partition groups."""
    n = a.shape[0]
    assert n % 16 == 0
    out = np.zeros((128, n // 16), np.int16)
    for c in range(8):
        out[16 * c : 16 * c + 16, :] = a.reshape(n // 16, 16).T
    return out


def _host_prep(inputs):
    import ml_dtypes

    bf16 = ml_dtypes.bfloat16
    f32 = np.float32
    feats = np.asarray(inputs["features"], f32)
    caps = np.asarray(inputs["captions"]).astype(np.int64)
    lens = np.asarray(inputs["lengths"]).reshape(-1).astype(np.int64)
    table = np.asarray(inputs["embed_table"], f32)

    W_ih = np.asarray(inputs["W_ih"], f32)
    W_hh = np.asarray(inputs["W_hh"], f32)
    b_ih = np.asarray(inputs["b_ih"], f32)
    b_hh = np.asarray(inputs["b_hh"], f32)
    b_beta = np.asarray(inputs["b_beta"], f32)
    b_fc = np.asarray(inputs["b_fc"], f32)
    b_hinit = np.asarray(inputs["b_hinit"], f32)
    b_cinit = np.asarray(inputs["b_cinit"], f32)

    # ragged-batch packing (lengths sorted descending by construction)
    b_t = [int((lens > t).sum()) for t in range(T)]
    off = np.concatenate([[0], np.cumsum(b_t)]).astype(np.int64)
    p_total = int(off[-1])
    p_pad = ((p_total + 127) // 128) * 128

    # bf16 table with zero rows closing the lo/hi split:
    #   rows 0..SPLIT-1   = table[:SPLIT],  row SPLIT = 0   (lo zero)
    #   rows SPLIT+1..    = table[SPLIT:],  last row  = 0   (hi zero)
    tableg = np.zeros((V + 2, H), bf16)
    tableg[:SPLIT] = table[:SPLIT].astype(bf16)
    tableg[SPLIT + 1 : V + 1] = table[SPLIT:].astype(bf16)

    # gather indices, t-major (t=1..19); 64 pad rows -> zero rows
    idx_flat = np.full(EMB_BLOCKS * 128, SPLIT, np.int64)
    idx_flat[:N_GATHER] = caps.T.reshape(-1)
    is_hi = idx_flat >= SPLIT
    idx_lo = np.where(is_hi, SPLIT, idx_flat).astype(np.int16)
    idx_hi = np.where(is_hi, idx_flat - SPLIT, Z_HI).astype(np.int16)

    w2emb = W_ih.T[:H]                      # [512, 2048] emb input rows
    w2ah = np.vstack([W_ih.T[H:], W_hh.T])  # [1024, 2048] awe+h input rows

    b2 = b_ih + b_hh
    has_b2 = bool(np.any(b2))
    has_bbeta = bool(np.any(b_beta))
    has_bfc = bool(np.any(b_fc))
    has_binit = bool(np.any(b_hinit)) or bool(np.any(b_cinit))

    common = {
        "tableg": tableg,
        "featT": _pack_k(feats.T.astype(f32)).astype(bf16),
        "featsb": feats.astype(bf16),
        "w2e": _pack_k(w2emb).astype(bf16),
        "w2ah": _pack_k(w2ah).astype(bf16),
        "wbeta": _pack_k(np.asarray(inputs["W_beta"], f32)).astype(bf16),
        "whinit": _pack_k(np.asarray(inputs["W_hinit"], f32)).astype(bf16),
        "wcinit": _pack_k(np.asarray(inputs["W_cinit"], f32)).astype(bf16),
        "ident": np.eye(128, dtype=f32).astype(bf16),
    }
    idh = np.zeros((128, 64), f32)
    for i in range(64):
        idh[64 + i, i] = 1.0
    common["identhi"] = idh.astype(bf16)
    for g, (a, b) in enumerate(GATHER_GROUPS):
        common[f"idxlo{g}"] = _pack_idx(idx_lo[a:b])
        common[f"idxhi{g}"] = _pack_idx(idx_hi[a:b])
    if has_b2:
        common["b2rep"] = np.ascontiguousarray(
            np.tile(b2[None, :], (128, 1)).astype(f32)
        )
    if has_bbeta or has_binit:
        common["ones"] = np.ones((1, B), bf16)
    if has_bbeta:
        common["bbetarow"] = b_beta.reshape(1, H).astype(bf16)
    if has_binit:
        common["bhinitT"] = np.ascontiguousarray(
            b_hinit.reshape(KC_H, 128).T.astype(f32)
        )
        common["bcinitrow"] = b_cinit.reshape(1, H).astype(bf16)

    W_fc = np.asarray(inputs["W_fc"], f32)
    wfc_pad = np.zeros((H, VPAD), f32)
    wfc_pad[:, :V] = W_fc
    bfc_pad = np.zeros(VPAD, f32)
    bfc_pad[:V] = b_fc

    in_maps = []
    for k in range(NCORES):
        m = dict(common)
        m["wfc"] = _pack_k(wfc_pad[:, k * VS : (k + 1) * VS]).astype(bf16)
        if has_bfc:
            m["bfcrep"] = np.ascontiguousarray(
                np.tile(bfc_pad[k * VS : (k + 1) * VS][None, :], (128, 1))
            ).astype(f32)
        in_maps.append(m)

    meta = {
        "b_t": b_t, "off": off, "p_total": p_total, "p_pad": p_pad,
        "has_b2": has_b2, "has_bbeta": has_bbeta, "has_bfc": has_bfc,
        "has_binit": has_binit,
    }
    return in_maps, meta


def build_program(meta):
    """Build the (SPMD-identical) Bass program. Per-core differences are data
    only (wfc shards)."""
    b_t = meta["b_t"]
    off = [int(x) for x in meta["off"]]
    p_total = meta["p_total"]
    p_pad = meta["p_pad"]
    mv = p_pad // 128
    has_b2 = meta["has_b2"]
    has_bbeta = meta["has_bbeta"]
    has_bfc = meta["has_bfc"]
    has_binit = meta["has_binit"]

    nc = bacc.Bacc(num_swdge_queues=4)

    tableg_d = nc.declare_dram_parameter("tableg", [V + 2, H], BF16, isOutput=False)
    featT_d = nc.declare_dram_parameter("featT", [128, KC_H, B], BF16, isOutput=False)
    featsb_d = nc.declare_dram_parameter("featsb", [B, H], BF16, isOutput=False)
    w2e_d = nc.declare_dram_parameter("w2e", [128, KC_H, GATE_N], BF16, isOutput=False)
    w2ah_d = nc.declare_dram_parameter("w2ah", [128, 8, GATE_N], BF16, isOutput=False)
    wbeta_d = nc.declare_dram_parameter("wbeta", [128, KC_H, H], BF16, isOutput=False)
    whinit_d = nc.declare_dram_parameter("whinit", [128, KC_H, H], BF16, isOutput=False)
    wcinit_d = nc.declare_dram_parameter("wcinit", [128, KC_H, H], BF16, isOutput=False)
    ident_d = nc.declare_dram_parameter("ident", [128, 128], BF16, isOutput=False)
    identhi_d = nc.declare_dram_parameter("identhi", [128, 64], BF16, isOutput=False)
    idx_d = {}
    for g, (a, b) in enumerate(GATHER_GROUPS):
        w = (b - a) // 16
        idx_d[("lo", g)] = nc.declare_dram_parameter(f"idxlo{g}", [128, w], I16, isOutput=False)
        idx_d[("hi", g)] = nc.declare_dram_parameter(f"idxhi{g}", [128, w], I16, isOutput=False)
    wfc_d = nc.declare_dram_parameter("wfc", [128, KC_H, VS], BF16, isOutput=False)
    if has_b2:
        b2rep_d = nc.declare_dram_parameter("b2rep", [128, GATE_N], F32, isOutput=False)
    if has_bbeta or has_binit:
        ones_d = nc.declare_dram_parameter("ones", [1, B], BF16, isOutput=False)
    if has_bbeta:
        bbetarow_d = nc.declare_dram_parameter("bbetarow", [1, H], BF16, isOutput=False)
    if has_binit:
        bhinitT_d = nc.declare_dram_parameter("bhinitT", [128, KC_H], F32, isOutput=False)
        bcinitrow_d = nc.declare_dram_parameter("bcinitrow", [1, H], BF16, isOutput=False)
    if has_bfc:
        bfcrep_d = nc.declare_dram_parameter("bfcrep", [128, VS], F32, isOutput=False)
    out_d = nc.declare_dram_parameter("out", [p_pad, VS], F32, isOutput=True)

    def mm(out, lhsT, rhs, start, stop):
        nc.tensor.matmul(out, lhsT, rhs, start=start, stop=stop)

    # vocab work item (mc, vo, vw) is runnable once all hall rows of block mc
    # are written, i.e. after step mc_ready[mc]'s h transposes.
    mc_ready = []
    for mc in range(mv):
        need = (mc + 1) * 128
        r = T - 1
        for t in range(T):
            if off[t + 1] >= need:
                r = t
                break
        mc_ready.append(r)
    vqueue = [(mc, vo, vw) for mc in range(mv) for (vo, vw) in VCHUNKS]

    # per-step hall write segments: (mc, local_lo, src_lo, n)
    hall_segs = []
    for t in range(T):
        segs = []
        lo, n = off[t], b_t[t]
        while n > 0:
            mc = lo // 128
            ll = lo % 128
            take = min(128 - ll, n)
            segs.append((mc, ll, lo - off[t], take))
            lo += take
            n -= take
        hall_segs.append(segs)

    SIG = mybir.ActivationFunctionType.Sigmoid
    TANH = mybir.ActivationFunctionType.Tanh

    with tile.TileContext(nc) as tc:
        nc.gpsimd.load_library(library_config.mlp)
        with (
            tc.tile_pool(name="const", bufs=1) as constp,
            tc.tile_pool(name="res", bufs=1) as resp,
            tc.tile_pool(name="state", bufs=1) as statep,
            tc.tile_pool(name="step", bufs=1) as sp,
            tc.tile_pool(name="stage", bufs=3) as stp,
            tc.tile_pool(name="gpsum", bufs=1, space="PSUM") as gps,
            tc.tile_pool(name="fill", bufs=2, space="PSUM") as fillp,
            tc.tile_pool(name="tpsum", bufs=2, space="PSUM") as tps,
        ):
            # --- tiny constants first so their DMAs head the queues ---
            ident = constp.tile([128, 128], BF16)
            nc.sync.dma_start(ident[:], ident_d[:])
            identhi = constp.tile([128, 64], BF16)
            nc.sync.dma_start(identhi[:], identhi_d[:])
            featT = constp.tile([128, KC_H, B], BF16)
            nc.sync.dma_start(featT[:], featT_d[:])
            featsb = constp.tile([B, H], BF16)
            nc.sync.dma_start(featsb[:], featsb_d[:])
            if has_bbeta or has_binit:
                ones = constp.tile([1, B], BF16)
                nc.sync.dma_start(ones[:], ones_d[:])
            if has_bbeta:
                bbetarow = constp.tile([1, H], BF16)
                nc.sync.dma_start(bbetarow[:], bbetarow_d[:])

            # --- persistent state ---
            w2ah = resp.tile([128, 8, GATE_N], BF16)
            nc.scalar.dma_start(w2ah[:], w2ah_d[:])
            wbeta = resp.tile([128, KC_H, H], BF16)
            nc.scalar.dma_start(wbeta[:], wbeta_d[:])
            e_sb = resp.tile([128, EMB_BLOCKS, GATE_N], BF16, name="E")
            hall_t = [
                resp.tile([128, KC_H, 128], BF16, tag=f"hall{mc}", name=f"hall{mc}")
                for mc in range(mv)
            ]
            if p_pad > p_total:
                mc = p_total // 128
                nc.vector.memset(hall_t[mc][:, :, p_total % 128 :], 0.0)
                for m2 in range(mc + 1, mv):
                    nc.vector.memset(hall_t[m2][:], 0.0)
            if has_b2:
                b2rep = resp.tile([128, GATE_N], F32)
                nc.scalar.dma_start(b2rep[:], b2rep_d[:])
            if has_bfc:
                bfcrep = resp.tile([128, VS], F32)
                nc.scalar.dma_start(bfcrep[:], bfcrep_d[:])

            wfc = resp.tile([128, KC_H, VS], BF16)

            hT = statep.tile([128, KC_H, B], BF16)
            aweT = statep.tile([128, KC_H, B], BF16)
            c_st = statep.tile([B, H], F32)

            # gate PSUM: four independent 512-wide chunks (i, f, g, o)
            g_ps = [gps.tile([B, H], F32, tag=f"g{x}", name=f"g{x}")
                    for x in range(4)]

            # =============== helpers ===============
            vstate = {"q": 0, "alt": 0}

            def vocab_item():
                mc, vo, vw = vqueue[vstate["q"]]
                vstate["q"] += 1
                pv = fillp.tile([128, H], F32, tag="fill")
                for kc in range(KC_H):
                    mm(pv[:, :vw], hall_t[mc][:, kc, :], wfc[:, kc, vo : vo + vw],
                       start=(kc == 0), stop=(kc == KC_H - 1))
                st = stp.tile([128, H], F32, tag="st")
                if has_bfc:
                    nc.vector.tensor_add(st[:, :vw], pv[:, :vw],
                                         bfcrep[:, vo : vo + vw])
                elif vstate["alt"] == 0:
                    nc.scalar.copy(st[:, :vw], pv[:, :vw])
                else:
                    nc.vector.tensor_copy(st[:, :vw], pv[:, :vw])
                vstate["alt"] ^= 1
                nc.sync.dma_start(
                    out_d[mc * 128 : (mc + 1) * 128, vo : vo + vw], st[:, :vw]
                )

            def emit_vocab(t):
                n = 0
                while (vstate["q"] < len(vqueue) and n < VOCAB_CAP
                       and mc_ready[vqueue[vstate["q"]][0]] < t):
                    vocab_item()
                    n += 1

            ep_alt = [0]

            def e_pre(mb, lhsT_tile, lhsT_col0, half=None):
                """E block mb from transposed emb rows [128, KC_H, *] at
                lhsT_tile[:, kc, lhsT_col0:+w]. half: None=128 rows,
                'lo'/'hi' = 64-row half blocks."""
                r0, rn = (0, 128) if half is None else ((0, 64) if half == "lo" else (64, 128))
                w = rn - r0
                for nc4 in range(4):
                    pse = fillp.tile([128, H], F32, tag="fill")
                    for kc in range(KC_H):
                        mm(pse[r0:rn, :],
                           lhsT_tile[:, kc, lhsT_col0 + r0 : lhsT_col0 + r0 + w],
                           w2e[:, kc, nc4 * 512 : (nc4 + 1) * 512],
                           start=(kc == 0), stop=(kc == KC_H - 1))
                    dst = e_sb[r0:rn, mb, nc4 * 512 : (nc4 + 1) * 512]
                    if has_b2:
                        nc.vector.tensor_add(
                            dst, pse[r0:rn, :],
                            b2rep[r0:rn, nc4 * 512 : (nc4 + 1) * 512])
                    elif ep_alt[0] == 0:
                        nc.scalar.copy(dst, pse[r0:rn, :])
                    else:
                        nc.vector.tensor_copy(dst, pse[r0:rn, :])
                    ep_alt[0] ^= 1

            def e_load(t):
                """Start the gate accumulation groups with E_t (+ implicitly
                the biases, folded into E)."""
                m0 = FEAT_OFF if t == 0 else (t - 1) * B
                p0, gslc = m0 % 128, m0 // 128
                idx = ident[0:64, 0:64] if p0 == 0 else identhi[64:128, 0:64]
                for x in range(4):
                    mm(g_ps[x][:], idx, e_sb[p0 : p0 + B, gslc, x * 512 : (x + 1) * 512],
                       start=True, stop=False)

            def h_trans(t, h_new):
                """Transpose h_new (= h_{t+1}, preds source of step t) into hT
                and scatter its active prefix into the hall tiles."""
                for kc in range(KC_H):
                    tp = tps.tile([128, B], BF16, tag="tp")
                    nc.tensor.transpose(
                        tp[:, 0:B], h_new[:, kc * 128 : (kc + 1) * 128],
                        ident[0:B, 0:B])
                    nc.vector.tensor_copy(hT[:, kc, :], tp[:])
                    for (mc, ll, sl, seg_n) in hall_segs[t]:
                        nc.vector.tensor_copy(
                            hall_t[mc][:, kc, ll : ll + seg_n],
                            tp[:, sl : sl + seg_n])

            def beta_mms():
                """beta = h @ W_beta [+ b_beta] into PSUM; returns the tile."""
                betaps = fillp.tile([128, H], F32, tag="fill")
                for kc in range(KC_H):
                    mm(betaps[0:B, :], hT[:, kc, :], wbeta[:, kc, :],
                       start=(kc == 0), stop=(kc == KC_H - 1) and not has_bbeta)
                if has_bbeta:
                    mm(betaps[0:B, :], ones[:], bbetarow[:], start=False, stop=True)
                return betaps

            def h_part():
                """h contribution to the gates (E already loaded)."""
                for kc in range(KC_H):
                    for x in range(4):
                        mm(g_ps[x][:], hT[:, kc, :],
                           w2ah[:, 4 + kc, x * 512 : (x + 1) * 512],
                           start=False, stop=False)

            def sig_awe(betaps):
                """awe = sigmoid(beta) * features, transposed into aweT. The
                ACT/DVE work overlaps the h_part matmuls on the PE."""
                sigb = sp.tile([B, H], BF16, tag="sigb")
                nc.scalar.activation(sigb[:], betaps[0:B, :], SIG)
                aweb = sp.tile([B, H], BF16, tag="aweb")
                nc.vector.tensor_mul(aweb[:], sigb[:], featsb[:])
                for kc in range(KC_H):
                    tp = tps.tile([128, B], BF16, tag="tp")
                    nc.tensor.transpose(
                        tp[:, 0:B], aweb[:, kc * 128 : (kc + 1) * 128],
                        ident[0:B, 0:B])
                    nc.vector.tensor_copy(aweT[:, kc, :], tp[:])

            def awe_and_pointwise(t):
                """awe gate matmuls + LSTM pointwise; returns h_new tile."""
                # awe contribution, chunk-major f,i,o,g so f completes first
                for x in (1, 0, 3, 2):
                    for kc in range(KC_H):
                        mm(g_ps[x][:], aweT[:, kc, :],
                           w2ah[:, kc, x * 512 : (x + 1) * 512],
                           start=False, stop=(kc == KC_H - 1))
                # pointwise (PyTorch gate order i, f, g, o)
                sig_f = sp.tile([B, H], F32, tag="sig_f")
                nc.scalar.activation(sig_f[:], g_ps[1][:], SIG)
                sig_i = sp.tile([B, H], F32, tag="sig_i")
                nc.scalar.activation(sig_i[:], g_ps[0][:], SIG)
                sig_o = sp.tile([B, H], BF16, tag="sig_o")
                nc.scalar.activation(sig_o[:], g_ps[3][:], SIG)
                tanh_g = sp.tile([B, H], F32, tag="tanh_g")
                nc.scalar.activation(tanh_g[:], g_ps[2][:], TANH)
                nc.vector.tensor_mul(c_st[:], c_st[:], sig_f[:])
                t2 = sp.tile([B, H], F32, tag="t2")
                nc.vector.tensor_mul(t2[:], sig_i[:], tanh_g[:])
                nc.vector.tensor_add(c_st[:], c_st[:], t2[:])
                tanh_c = sp.tile([B, H], BF16, tag="tanh_c")
                nc.scalar.activation(tanh_c[:], c_st[:], TANH)
                h_new = sp.tile([B, H], BF16, tag="h_new")
                nc.vector.tensor_mul(h_new[:], sig_o[:], tanh_c[:])
                return h_new

            # ================= prep phase =================
            with (
                tc.tile_pool(name="prew", bufs=1) as prew,
                tc.tile_pool(name="gath", bufs=2) as gp,
            ):
                glo = [None] * 4
                ghi = [None] * 4

                def gather(g):
                    a, b = GATHER_GROUPS[g]
                    n = b - a
                    idxlo = gp.tile([128, 24], I16, tag="ilo")
                    idxhi = gp.tile([128, 24], I16, tag="ihi")
                    w = n // 16
                    nc.sync.dma_start(idxlo[:, :w], idx_d[("lo", g)][:])
                    nc.sync.dma_start(idxhi[:, :w], idx_d[("hi", g)][:])
                    if n == 384:
                        glo[g] = gp.tile([128, KC_H, 384], BF16, tag="glo",
                                         name=f"glo{g}")
                        ghi[g] = gp.tile([128, KC_H, 384], BF16, tag="ghi",
                                         name=f"ghi{g}")
                    else:
                        glo[g] = gp.tile([128, KC_H, n], BF16, tag=f"glo{g}",
                                         bufs=1, name=f"glo{g}")
                        ghi[g] = gp.tile([128, KC_H, n], BF16, tag=f"ghi{g}",
                                         bufs=1, name=f"ghi{g}")
                    nc.gpsimd.dma_gather(
                        glo[g][:, :, :n], tableg_d[0 : SPLIT + 1, :],
                        idxlo[:, :w], num_idxs=n, num_idxs_reg=n,
                        elem_size=H, transpose=True, queue_num=g % 4,
                    )
                    nc.gpsimd.dma_gather(
                        ghi[g][:, :, :n], tableg_d[SPLIT + 1 : V + 2, :],
                        idxhi[:, :w], num_idxs=n, num_idxs_reg=n,
                        elem_size=H, transpose=True, queue_num=g % 4,
                    )

                def blend(g):
                    a, b = GATHER_GROUPS[g]
                    n = b - a
                    nc.vector.tensor_add(
                        glo[g][:, :, :n], glo[g][:, :, :n], ghi[g][:, :, :n])

                gather(0)
                gather(1)

                w2e = prew.tile([128, KC_H, GATE_N], BF16, bufs=1)
                nc.scalar.dma_start(w2e[:], w2e_d[:])

                # h0 (transposed directly) and c0 while gathers fly
                with tc.tile_pool(name="initp", bufs=1) as ip:
                    whinit = ip.tile([128, KC_H, H], BF16)
                    nc.sync.dma_start(whinit[:], whinit_d[:])
                    wcinit = ip.tile([128, KC_H, H], BF16)
                    nc.sync.dma_start(wcinit[:], wcinit_d[:])
                    if has_binit:
                        bhinitT = ip.tile([128, KC_H], F32)
                        nc.sync.dma_start(bhinitT[:], bhinitT_d[:])
                        bcinitrow = ip.tile([1, H], BF16)
                        nc.sync.dma_start(bcinitrow[:], bcinitrow_d[:])
                    for jb in range(KC_H):
                        hps = fillp.tile([128, H], F32, tag="fill")
                        for kc in range(KC_H):
                            mm(hps[:, 0:B], whinit[:, kc, jb * 128 : (jb + 1) * 128],
                               featT[:, kc, :], start=(kc == 0), stop=(kc == KC_H - 1))
                        if has_binit:
                            nc.scalar.activation(
                                hT[:, jb, :], hps[:, 0:B],
                                mybir.ActivationFunctionType.Identity,
                                bias=bhinitT[:, jb : jb + 1])
                        else:
                            nc.vector.tensor_copy(hT[:, jb, :], hps[:, 0:B])
                    cps = fillp.tile([128, H], F32, tag="fill")
                    for kc in range(KC_H):
                        mm(cps[0:B, :], featT[:, kc, :], wcinit[:, kc, :],
                           start=(kc == 0), stop=(kc == KC_H - 1) and not has_binit)
                    if has_binit:
                        mm(cps[0:B, :], ones[:], bcinitrow[:], start=False, stop=True)
                    nc.vector.tensor_copy(c_st[:], cps[0:B, :])

                # E for t=0 (features block = hi half of block 9; featT holds
                # exactly those 64 rows, hence the -64 column bias)
                e_pre(EMB_BLOCKS - 1, featT, -64, half="hi")

                # wfc DMA after the early weights (first vocab item is t>=3)
                nc.scalar.dma_start(wfc[:], wfc_d[:])

                # ---- step 0 ----
                e_load(0)
                bps = beta_mms()
                h_part()
                sig_awe(bps)
                h_new = awe_and_pointwise(0)

                # ---- steps 1..8 with interleaved gather/E work ----
                def step(t):
                    e_load(t)
                    h_trans(t - 1, h_new)
                    bps = beta_mms()
                    h_part()
                    sig_awe(bps)
                    hn = awe_and_pointwise(t)
                    emit_vocab(t)
                    return hn

                blend(0)
                for mb in (0, 1, 2):
                    e_pre(mb, glo[0], mb * 128)
                h_new = step(1)
                gather(2)
                h_new = step(2)
                blend(1)
                for mb in (3, 4, 5):
                    e_pre(mb, glo[1], (mb - 3) * 128)
                h_new = step(3)
                gather(3)
                h_new = step(4)
                blend(2)
                for mb in (6, 7, 8):
                    e_pre(mb, glo[2], (mb - 6) * 128)
                h_new = step(5)
                h_new = step(6)
                blend(3)
                e_pre(EMB_BLOCKS - 1, glo[3], 0, half="lo")
                h_new = step(7)
                h_new = step(8)

            # ============ steady recurrence ============
            for t in range(9, T):
                h_new = step(t)

            h_trans(T - 1, h_new)
            while vstate["q"] < len(vqueue):
                vocab_item()

    nc.finalize()
    return nc


def kernel(**inputs):
    in_maps, meta = _host_prep(inputs)
    nc = build_program(meta)
    res = run_bass_kernel_spmd(nc, in_maps, core_ids=list(range(NCORES)))
    results = res.results

    b_t = meta["b_t"]
    off = meta["off"]
    full = np.zeros((B, T, VPAD), np.float32)
    for k in range(NCORES):
        o = np.asarray(results[k]["out"])
        for t in range(T):
            bt = b_t[t]
            if bt:
                full[:bt, t, k * VS : (k + 1) * VS] = o[off[t] : off[t] + bt]
    return full[:, :, :V]


# revision 15
# speedup vs baseline: 1.3086x; 1.0431x over previous
"""Trainium2 Bass kernel for an attention-LSTM caption decoder.

Math notes (verified against the reference on CPU):
  - num_pixels == 1 makes the softmax attention exactly a no-op: alpha == 1.0,
    so awe = sigmoid(h @ W_beta) * features. W_enc/W_dec/W_full are unused.
  - Masked (b, t) rows (t >= len[b]) never re-activate and never influence
    active rows, so h/c freezing can be dropped; only output masking matters.
    We compute/emit only the active rows (lengths are sorted descending, so
    the active rows at step t are a prefix of the batch).
  - All biases in setup_inputs() are zero; detected host-side, bias work is
    compiled out (fallback paths are kept for nonzero biases).

Distribution (8 cores): recurrence replicated on every core; fc weight and
the output vocab dim sharded 8-way (tensor parallel). The embedding table
(bf16) is resident in each core's HBM.

Schedule highlights vs the v0 kernel:
  - The embedding gather uses dma_gather(transpose=True) on a bf16 table,
    landing rows directly in the transposed [128, kc, row] layout; a zero
    row appended to each table half turns the lo/hi split into a plain add
    (no select mask). 4 swdge queues, gathers issued before the big weight
    DMAs so they overlap.
  - E = emb @ W_ih_emb.T is kept in SBUF (no DRAM staging) and its blocks
    are computed interleaved with the first recurrence steps.
  - The LSTM gate PSUM is split into four independent 512-wide tiles so the
    pointwise chain on chunk f starts while chunks i/o/g are still being
    accumulated on the PE (awe matmuls are emitted f,i,o,g).
  - Vocab-projection matmuls fill the PE idle window during the pointwise
    chain; their PSUM->SBUF copies alternate between ScalarE and VectorE.
"""

import numpy as np

from concourse import bacc, bass, library_config, mybir, tile
from concourse.bass_utils import run_bass_kernel_spmd

F32 = mybir.dt.float32
BF16 = mybir.dt.bfloat16
I16 = mybir.dt.int16

B = 64
H = 512
T = 20
V = 50257
NCORES = 8
VS = 6284            # per-core vocab shard (8 * 6284 = 50272 >= 50257)
VPAD = NCORES * VS
KC_H = H // 128      # 4 k-chunks per 512-wide contraction
GATE_N = 4 * H       # 2048
SPLIT = 32000        # embedding table split (int16 index range)
NHI = V - SPLIT      # 18257 hi rows
Z_HI = NHI           # local index of the zero row in the hi half

N_GATHER = (T - 1) * B           # 1216 gathered rows, j = (t-1)*64 + b
EMB_BLOCKS = 10                  # E blocks of 128 rows (1280 incl features)
FEAT_OFF = N_GATHER              # features rows live at 1216..1279 (block 9 hi)
# 128-aligned groups: g0 small so steps 1-2 unblock as early as possible
GATHER_GROUPS = [(0, 128), (128, 512), (512, 896), (896, 1280)]
VCHUNKS = [(i * 512, min(512, VS - i * 512)) for i in range(13)]
VOCAB_CAP = 5                    # max vocab items interleaved per step


def _pack_k(w):
    """[K, N] -> [128, K//128, N] with the contraction dim on partitions."""
    k, n = w.shape
    assert k % 128 == 0
    return np.ascontiguousarray(w.reshape(k // 128, 128, n).transpose(1, 0, 2))


def _pack_idx(a):
    """(n,) int16 -> [128, n//16]; j = s*16 + p wrapping, replicated into each
    of the 8 GPSIMD Q7 cores' 16-partition groups."""
    n = a.shape[0]
    assert n % 16 == 0
    out = np.zeros((128, n // 16), np.int16)
    for c in range(8):
        out[16 * c : 16 * c + 16, :] = a.reshape(n // 16, 16).T
    return out


def _host_prep(inputs):
    import ml_dtypes

    bf16 = ml_dtypes.bfloat16
    f32 = np.float32
    feats = np.asarray(inputs["features"], f32)
    caps = np.asarray(inputs["captions"]).astype(np.int64)
    lens = np.asarray(inputs["lengths"]).reshape(-1).astype(np.int64)
    table = np.asarray(inputs["embed_table"], f32)

    W_ih = np.asarray(inputs["W_ih"], f32)
    W_hh = np.asarray(inputs["W_hh"], f32)
    b_ih = np.asarray(inputs["b_ih"], f32)
    b_hh = np.asarray(inputs["b_hh"], f32)
    b_beta = np.asarray(inputs["b_beta"], f32)
    b_fc = np.asarray(inputs["b_fc"], f32)
    b_hinit = np.asarray(inputs["b_hinit"], f32)
    b_cinit = np.asarray(inputs["b_cinit"], f32)

    # ragged-batch packing (lengths sorted descending by construction)
    b_t = [int((lens > t).sum()) for t in range(T)]
    off = np.concatenate([[0], np.cumsum(b_t)]).astype(np.int64)
    p_total = int(off[-1])
    p_pad = ((p_total + 127) // 128) * 128

    # bf16 table with zero rows closing the lo/hi split:
    #   rows 0..SPLIT-1   = table[:SPLIT],  row SPLIT = 0   (lo zero)
    #   rows SPLIT+1..    = table[SPLIT:],  last row  = 0   (hi zero)
    tableg = np.zeros((V + 2, H), bf16)
    tableg[:SPLIT] = table[:SPLIT].astype(bf16)
    tableg[SPLIT + 1 : V + 1] = table[SPLIT:].astype(bf16)

    # gather indices, t-major (t=1..19); 64 pad rows -> zero rows
    idx_flat = np.full(EMB_BLOCKS * 128, SPLIT, np.int64)
    idx_flat[:N_GATHER] = caps.T.reshape(-1)
    is_hi = idx_flat >= SPLIT
    idx_lo = np.where(is_hi, SPLIT, idx_flat).astype(np.int16)
    idx_hi = np.where(is_hi, idx_flat - SPLIT, Z_HI).astype(np.int16)

    w2emb = W_ih.T[:H]                      # [512, 2048] emb input rows
    w2ah = np.vstack([W_ih.T[H:], W_hh.T])  # [1024, 2048] awe+h input rows

    b2 = b_ih + b_hh
    has_b2 = bool(np.any(b2))
    has_bbeta = bool(np.any(b_beta))
    has_bfc = bool(np.any(b_fc))
    has_binit = bool(np.any(b_hinit)) or bool(np.any(b_cinit))

    common = {
        "tableg": tableg,
        "featT": _pack_k(feats.T.astype(f32)).astype(bf16),
        "featsb": feats.astype(bf16),
        "w2e": _pack_k(w2emb).astype(bf16),
        "w2ah": _pack_k(w2ah).astype(bf16),
        "wbeta": _pack_k(np.asarray(inputs["W_beta"], f32)).astype(bf16),
        "whinit": _pack_k(np.asarray(inputs["W_hinit"], f32)).astype(bf16),
        "wcinit": _pack_k(np.asarray(inputs["W_cinit"], f32)).astype(bf16),
        "ident": np.eye(128, dtype=f32).astype(bf16),
    }
    idh = np.zeros((128, 64), f32)
    for i in range(64):
        idh[64 + i, i] = 1.0
    common["identhi"] = idh.astype(bf16)
    for g, (a, b) in enumerate(GATHER_GROUPS):
        common[f"idxlo{g}"] = _pack_idx(idx_lo[a:b])
        common[f"idxhi{g}"] = _pack_idx(idx_hi[a:b])
    if has_b2:
        common["b2rep"] = np.ascontiguousarray(
            np.tile(b2[None, :], (128, 1)).astype(f32)
        )
    if has_bbeta or has_binit:
        common["ones"] = np.ones((1, B), bf16)
    if has_bbeta:
        common["bbetarow"] = b_beta.reshape(1, H).astype(bf16)
    if has_binit:
        common["bhinitT"] = np.ascontiguousarray(
            b_hinit.reshape(KC_H, 128).T.astype(f32)
        )
        common["bcinitrow"] = b_cinit.reshape(1, H).astype(bf16)

    W_fc = np.asarray(inputs["W_fc"], f32)
    wfc_pad = np.zeros((H, VPAD), f32)
    wfc_pad[:, :V] = W_fc
    bfc_pad = np.zeros(VPAD, f32)
    bfc_pad[:V] = b_fc

    in_maps = []
    for k in range(NCORES):
        m = dict(common)
        m["wfc"] = _pack_k(wfc_pad[:, k * VS : (k + 1) * VS]).astype(bf16)
        if has_bfc:
            m["bfcrep"] = np.ascontiguousarray(
                np.tile(bfc_pad[k * VS : (k + 1) * VS][None, :], (128, 1))
            ).astype(f32)
        in_maps.append(m)

    meta = {
        "b_t": b_t, "off": off, "p_total": p_total, "p_pad": p_pad,
        "has_b2": has_b2, "has_bbeta": has_bbeta, "has_bfc": has_bfc,
        "has_binit": has_binit,
    }
    return in_maps, meta


def build_program(meta):
    """Build the (SPMD-identical) Bass program. Per-core differences are data
    only (wfc shards)."""
    b_t = meta["b_t"]
    off = [int(x) for x in meta["off"]]
    p_total = meta["p_total"]
    p_pad = meta["p_pad"]
    mv = p_pad // 128
    has_b2 = meta["has_b2"]
    has_bbeta = meta["has_bbeta"]
    has_bfc = meta["has_bfc"]
    has_binit = meta["has_binit"]

    nc = bacc.Bacc(num_swdge_queues=4)

    tableg_d = nc.declare_dram_parameter("tableg", [V + 2, H], BF16, isOutput=False)
    featT_d = nc.declare_dram_parameter("featT", [128, KC_H, B], BF16, isOutput=False)
    featsb_d = nc.declare_dram_parameter("featsb", [B, H], BF16, isOutput=False)
    w2e_d = nc.declare_dram_parameter("w2e", [128, KC_H, GATE_N], BF16, isOutput=False)
    w2ah_d = nc.declare_dram_parameter("w2ah", [128, 8, GATE_N], BF16, isOutput=False)
    wbeta_d = nc.declare_dram_parameter("wbeta", [128, KC_H, H], BF16, isOutput=False)
    whinit_d = nc.declare_dram_parameter("whinit", [128, KC_H, H], BF16, isOutput=False)
    wcinit_d = nc.declare_dram_parameter("wcinit", [128, KC_H, H], BF16, isOutput=False)
    ident_d = nc.declare_dram_parameter("ident", [128, 128], BF16, isOutput=False)
    identhi_d = nc.declare_dram_parameter("identhi", [128, 64], BF16, isOutput=False)
    idx_d = {}
    for g, (a, b) in enumerate(GATHER_GROUPS):
        w = (b - a) // 16
        idx_d[("lo", g)] = nc.declare_dram_parameter(f"idxlo{g}", [128, w], I16, isOutput=False)
        idx_d[("hi", g)] = nc.declare_dram_parameter(f"idxhi{g}", [128, w], I16, isOutput=False)
    wfc_d = nc.declare_dram_parameter("wfc", [128, KC_H, VS], BF16, isOutput=False)
    if has_b2:
        b2rep_d = nc.declare_dram_parameter("b2rep", [128, GATE_N], F32, isOutput=False)
    if has_bbeta or has_binit:
        ones_d = nc.declare_dram_parameter("ones", [1, B], BF16, isOutput=False)
    if has_bbeta:
        bbetarow_d = nc.declare_dram_parameter("bbetarow", [1, H], BF16, isOutput=False)
    if has_binit:
        bhinitT_d = nc.declare_dram_parameter("bhinitT", [128, KC_H], F32, isOutput=False)
        bcinitrow_d = nc.declare_dram_parameter("bcinitrow", [1, H], BF16, isOutput=False)
    if has_bfc:
        bfcrep_d = nc.declare_dram_parameter("bfcrep", [128, VS], F32, isOutput=False)
    out_d = nc.declare_dram_parameter("out", [p_pad, VS], F32, isOutput=True)

    def mm(out, lhsT, rhs, start, stop):
        nc.tensor.matmul(out, lhsT, rhs, start=start, stop=stop)

    # vocab work item (mc, vo, vw) is runnable once all hall rows of block mc
    # are written, i.e. after step mc_ready[mc]'s h transposes.
    mc_ready = []
    for mc in range(mv):
        need = (mc + 1) * 128
        r = T - 1
        for t in range(T):
            if off[t + 1] >= need:
                r = t
                break
        mc_ready.append(r)
    vqueue = [(mc, vo, vw) for mc in range(mv) for (vo, vw) in VCHUNKS]

    # per-step hall write segments: (mc, local_lo, src_lo, n)
    hall_segs = []
    for t in range(T):
        segs = []
        lo, n = off[t], b_t[t]
        while n > 0:
            mc = lo // 128
            ll = lo % 128
            take = min(128 - ll, n)
            segs.append((mc, ll, lo - off[t], take))
            lo += take
            n -= take
        hall_segs.append(segs)

    SIG = mybir.ActivationFunctionType.Sigmoid
    TANH = mybir.ActivationFunctionType.Tanh

    with tile.TileContext(nc) as tc:
        nc.gpsimd.load_library(library_config.mlp)
        with (
            tc.tile_pool(name="const", bufs=1) as constp,
            tc.tile_pool(name="res", bufs=1) as resp,
            tc.tile_pool(name="state", bufs=1) as statep,
            tc.tile_pool(name="step", bufs=1) as sp,
            tc.tile_pool(name="stage", bufs=3) as stp,
            tc.tile_pool(name="gpsum", bufs=1, space="PSUM") as gps,
            tc.tile_pool(name="fill", bufs=2, space="PSUM") as fillp,
            tc.tile_pool(name="tpsum", bufs=2, space="PSUM") as tps,
        ):
            # --- tiny constants first so their DMAs head the queues ---
            ident = constp.tile([128, 128], BF16)
            nc.sync.dma_start(ident[:], ident_d[:])
            identhi = constp.tile([128, 64], BF16)
            nc.sync.dma_start(identhi[:], identhi_d[:])
            featT = constp.tile([128, KC_H, B], BF16)
            nc.sync.dma_start(featT[:], featT_d[:])
            featsb = constp.tile([B, H], BF16)
            nc.sync.dma_start(featsb[:], featsb_d[:])
            if has_bbeta or has_binit:
                ones = constp.tile([1, B], BF16)
                nc.sync.dma_start(ones[:], ones_d[:])
            if has_bbeta:
                bbetarow = constp.tile([1, H], BF16)
                nc.sync.dma_start(bbetarow[:], bbetarow_d[:])

            # --- persistent state (DMAs started after the gathers) ---
            w2ah = resp.tile([128, 8, GATE_N], BF16)
            wbeta = resp.tile([128, KC_H, H], BF16)
            e_sb = resp.tile([128, EMB_BLOCKS, GATE_N], BF16, name="E")
            hall_t = [
                resp.tile([128, KC_H, 128], BF16, tag=f"hall{mc}", name=f"hall{mc}")
                for mc in range(mv)
            ]
            if p_pad > p_total:
                mc = p_total // 128
                nc.vector.memset(hall_t[mc][:, :, p_total % 128 :], 0.0)
                for m2 in range(mc + 1, mv):
                    nc.vector.memset(hall_t[m2][:], 0.0)
            if has_b2:
                b2rep = resp.tile([128, GATE_N], F32)
            if has_bfc:
                bfcrep = resp.tile([128, VS], F32)

            wfc = resp.tile([128, KC_H, VS], BF16)

            hT = statep.tile([128, KC_H, B], BF16)
            aweT = statep.tile([128, KC_H, B], BF16)
            c_st = statep.tile([B, H], F32)

            # gate PSUM: four independent 512-wide chunks (i, f, g, o)
            g_ps = [gps.tile([B, H], F32, tag=f"g{x}", name=f"g{x}")
                    for x in range(4)]

            # =============== helpers ===============
            vstate = {"q": 0, "alt": 0}

            def vocab_item():
                mc, vo, vw = vqueue[vstate["q"]]
                vstate["q"] += 1
                pv = fillp.tile([128, H], F32, tag="fill")
                for kc in range(KC_H):
                    mm(pv[:, :vw], hall_t[mc][:, kc, :], wfc[:, kc, vo : vo + vw],
                       start=(kc == 0), stop=(kc == KC_H - 1))
                st = stp.tile([128, H], F32, tag="st")
                if has_bfc:
                    nc.vector.tensor_add(st[:, :vw], pv[:, :vw],
                                         bfcrep[:, vo : vo + vw])
                elif vstate["alt"] == 0:
                    nc.scalar.copy(st[:, :vw], pv[:, :vw])
                else:
                    nc.vector.tensor_copy(st[:, :vw], pv[:, :vw])
                vstate["alt"] ^= 1
                nc.sync.dma_start(
                    out_d[mc * 128 : (mc + 1) * 128, vo : vo + vw], st[:, :vw]
                )

            def emit_vocab(t):
                n = 0
                while (vstate["q"] < len(vqueue) and n < VOCAB_CAP
                       and mc_ready[vqueue[vstate["q"]][0]] < t):
                    vocab_item()
                    n += 1

            ep_alt = [0]

            def e_pre(mb, lhsT_tile, lhsT_col0, half=None):
                """E block mb from transposed emb rows [128, KC_H, *] at
                lhsT_tile[:, kc, lhsT_col0:+w]. half: None=128 rows,
                'lo'/'hi' = 64-row half blocks."""
                r0, rn = (0, 128) if half is None else ((0, 64) if half == "lo" else (64, 128))
                w = rn - r0
                for nc4 in range(4):
                    pse = fillp.tile([128, H], F32, tag="fill")
                    for kc in range(KC_H):
                        mm(pse[r0:rn, :],
                           lhsT_tile[:, kc, lhsT_col0 + r0 : lhsT_col0 + r0 + w],
                           w2e[:, kc, nc4 * 512 : (nc4 + 1) * 512],
                           start=(kc == 0), stop=(kc == KC_H - 1))
                    dst = e_sb[r0:rn, mb, nc4 * 512 : (nc4 + 1) * 512]
                    if has_b2:
                        nc.vector.tensor_add(
                            dst, pse[r0:rn, :],
                            b2rep[r0:rn, nc4 * 512 : (nc4 + 1) * 512])
                    elif ep_alt[0] == 0:
                        nc.scalar.copy(dst, pse[r0:rn, :])
                    else:
                        nc.vector.tensor_copy(dst, pse[r0:rn, :])
                    ep_alt[0] ^= 1

            def e_load(t):
                """Start the gate accumulation groups with E_t (+ implicitly
                the biases, folded into E)."""
                m0 = FEAT_OFF if t == 0 else (t - 1) * B
                p0, gslc = m0 % 128, m0 // 128
                idx = ident[0:64, 0:64] if p0 == 0 else identhi[64:128, 0:64]
                for x in range(4):
                    mm(g_ps[x][:], idx, e_sb[p0 : p0 + B, gslc, x * 512 : (x + 1) * 512],
                       start=True, stop=False)

            def h_trans(t, h_new):
                """Transpose h_new (= h_{t+1}, preds source of step t) into hT
                and scatter its active prefix into the hall tiles."""
                for kc in range(KC_H):
                    tp = tps.tile([128, B], BF16, tag="tp")
                    nc.tensor.transpose(
                        tp[:, 0:B], h_new[:, kc * 128 : (kc + 1) * 128],
                        ident[0:B, 0:B])
                    nc.vector.tensor_copy(hT[:, kc, :], tp[:])
                    for (mc, ll, sl, seg_n) in hall_segs[t]:
                        nc.vector.tensor_copy(
                            hall_t[mc][:, kc, ll : ll + seg_n],
                            tp[:, sl : sl + seg_n])

            def beta_mms():
                """beta = h @ W_beta [+ b_beta] into PSUM; returns the tile."""
                betaps = fillp.tile([128, H], F32, tag="fill")
                for kc in range(KC_H):
                    mm(betaps[0:B, :], hT[:, kc, :], wbeta[:, kc, :],
                       start=(kc == 0), stop=(kc == KC_H - 1) and not has_bbeta)
                if has_bbeta:
                    mm(betaps[0:B, :], ones[:], bbetarow[:], start=False, stop=True)
                return betaps

            def h_part():
                """h contribution to the gates (E already loaded)."""
                for kc in range(KC_H):
                    for x in range(4):
                        mm(g_ps[x][:], hT[:, kc, :],
                           w2ah[:, 4 + kc, x * 512 : (x + 1) * 512],
                           start=False, stop=False)

            def sig_awe(betaps):
                """awe = sigmoid(beta) * features, transposed into aweT. The
                ACT/DVE work overlaps the h_part matmuls on the PE."""
                sigb = sp.tile([B, H], BF16, tag="sigb")
                nc.scalar.activation(sigb[:], betaps[0:B, :], SIG)
                aweb = sp.tile([B, H], BF16, tag="aweb")
                nc.vector.tensor_mul(aweb[:], sigb[:], featsb[:])
                for kc in range(KC_H):
                    tp = tps.tile([128, B], BF16, tag="tp")
                    nc.tensor.transpose(
                        tp[:, 0:B], aweb[:, kc * 128 : (kc + 1) * 128],
                        ident[0:B, 0:B])
                    nc.vector.tensor_copy(aweT[:, kc, :], tp[:])

            def awe_and_pointwise(t):
                """awe gate matmuls + LSTM pointwise; returns h_new tile."""
                # awe contribution, chunk-major f,i,o,g so f completes first
                for x in (1, 0, 3, 2):
                    for kc in range(KC_H):
                        mm(g_ps[x][:], aweT[:, kc, :],
                           w2ah[:, kc, x * 512 : (x + 1) * 512],
                           start=False, stop=(kc == KC_H - 1))
                # pointwise (PyTorch gate order i, f, g, o)
                sig_f = sp.tile([B, H], F32, tag="sig_f")
                nc.scalar.activation(sig_f[:], g_ps[1][:], SIG)
                sig_i = sp.tile([B, H], F32, tag="sig_i")
                nc.scalar.activation(sig_i[:], g_ps[0][:], SIG)
                sig_o = sp.tile([B, H], BF16, tag="sig_o")
                nc.scalar.activation(sig_o[:], g_ps[3][:], SIG)
                tanh_g = sp.tile([B, H], F32, tag="tanh_g")
                nc.scalar.activation(tanh_g[:], g_ps[2][:], TANH)
                nc.vector.tensor_mul(c_st[:], c_st[:], sig_f[:])
                t2 = sp.tile([B, H], F32, tag="t2")
                nc.vector.tensor_mul(t2[:], sig_i[:], tanh_g[:])
                nc.vector.tensor_add(c_st[:], c_st[:], t2[:])
                tanh_c = sp.tile([B, H], BF16, tag="tanh_c")
                nc.scalar.activation(tanh_c[:], c_st[:], TANH)
                h_new = sp.tile([B, H], BF16, tag="h_new")
                nc.vector.tensor_mul(h_new[:], sig_o[:], tanh_c[:])
                return h_new

            # ================= prep phase =================
            with (
                tc.tile_pool(name="prew", bufs=1) as prew,
                tc.tile_pool(name="gath", bufs=2) as gp,
            ):
                glo = [None] * 4
                ghi = [None] * 4

                def gather(g):
                    a, b = GATHER_GROUPS[g]
                    n = b - a
                    w = n // 16
                    idxlo = gp.tile([128, w], I16, tag=f"ilo{g}", bufs=1,
                                    name=f"ilo{g}")
                    idxhi = gp.tile([128, w], I16, tag=f"ihi{g}", bufs=1,
                                    name=f"ihi{g}")
                    nc.sync.dma_start(idxlo[:], idx_d[("lo", g)][:])
                    nc.sync.dma_start(idxhi[:], idx_d[("hi", g)][:])
                    glo[g] = gp.tile([128, KC_H, n], BF16, tag=f"glo{g}",
                                     bufs=1, name=f"glo{g}")
                    ghi[g] = gp.tile([128, KC_H, n], BF16, tag=f"ghi{g}",
                                     bufs=1, name=f"ghi{g}")
                    nc.gpsimd.dma_gather(
                        glo[g][:], tableg_d[0 : SPLIT + 1, :],
                        idxlo[:], num_idxs=n, num_idxs_reg=n,
                        elem_size=H, transpose=True, queue_num=0,
                    )
                    nc.gpsimd.dma_gather(
                        ghi[g][:], tableg_d[SPLIT + 1 : V + 2, :],
                        idxhi[:], num_idxs=n, num_idxs_reg=n,
                        elem_size=H, transpose=True, queue_num=0,
                    )

                def blend(g):
                    a, b = GATHER_GROUPS[g]
                    n = b - a
                    nc.vector.tensor_add(glo[g][:], glo[g][:], ghi[g][:])

                # gathers first: their DMAs race ahead of the weight loads
                for g in range(4):
                    gather(g)

                # warm the sigmoid/tanh ACT table during the DMA wait
                warm = sp.tile([1, 2], F32, tag="warm")
                nc.scalar.activation(warm[:], ident[0:1, 0:2], SIG)

                # weight DMAs, ordered by first use
                nc.scalar.dma_start(wbeta[:], wbeta_d[:])
                w2e = prew.tile([128, KC_H, GATE_N], BF16, bufs=1)
                nc.scalar.dma_start(w2e[:], w2e_d[:])
                nc.scalar.dma_start(w2ah[:], w2ah_d[:])
                if has_b2:
                    nc.scalar.dma_start(b2rep[:], b2rep_d[:])
                if has_bfc:
                    nc.scalar.dma_start(bfcrep[:], bfcrep_d[:])

                # h0 (transposed directly) and c0 while gathers fly
                with tc.tile_pool(name="initp", bufs=1) as ip:
                    whinit = ip.tile([128, KC_H, H], BF16)
                    nc.sync.dma_start(whinit[:], whinit_d[:])
                    wcinit = ip.tile([128, KC_H, H], BF16)
                    nc.sync.dma_start(wcinit[:], wcinit_d[:])
                    if has_binit:
                        bhinitT = ip.tile([128, KC_H], F32)
                        nc.sync.dma_start(bhinitT[:], bhinitT_d[:])
                        bcinitrow = ip.tile([1, H], BF16)
                        nc.sync.dma_start(bcinitrow[:], bcinitrow_d[:])
                    for jb in range(KC_H):
                        hps = fillp.tile([128, H], F32, tag="fill")
                        for kc in range(KC_H):
                            mm(hps[:, 0:B], whinit[:, kc, jb * 128 : (jb + 1) * 128],
                               featT[:, kc, :], start=(kc == 0), stop=(kc == KC_H - 1))
                        if has_binit:
                            nc.scalar.activation(
                                hT[:, jb, :], hps[:, 0:B],
                                mybir.ActivationFunctionType.Identity,
                                bias=bhinitT[:, jb : jb + 1])
                        else:
                            nc.vector.tensor_copy(hT[:, jb, :], hps[:, 0:B])
                    cps = fillp.tile([128, H], F32, tag="fill")
                    for kc in range(KC_H):
                        mm(cps[0:B, :], featT[:, kc, :], wcinit[:, kc, :],
                           start=(kc == 0), stop=(kc == KC_H - 1) and not has_binit)
                    if has_binit:
                        mm(cps[0:B, :], ones[:], bcinitrow[:], start=False, stop=True)
                    nc.vector.tensor_copy(c_st[:], cps[0:B, :])

                # E for t=0 (features block = hi half of block 9; featT holds
                # exactly those 64 rows, hence the -64 column bias)
                e_pre(EMB_BLOCKS - 1, featT, -64, half="hi")

                # wfc DMA after the early weights (first vocab item is t>=3)
                nc.scalar.dma_start(wfc[:], wfc_d[:])

                # ---- step 0 ----
                e_load(0)
                bps = beta_mms()
                h_part()
                sig_awe(bps)
                h_new = awe_and_pointwise(0)

                # ---- steps 1..8 with interleaved gather/E work ----
                def step(t):
                    e_load(t)
                    h_trans(t - 1, h_new)
                    bps = beta_mms()
                    h_part()
                    sig_awe(bps)
                    hn = awe_and_pointwise(t)
                    emit_vocab(t)
                    return hn

                # E-block needs: step t reads block (t-1)//2
                blend(0)
                e_pre(0, glo[0], 0)
                h_new = step(1)
                h_new = step(2)
                blend(1)
                for mb in (1, 2, 3):
                    e_pre(mb, glo[1], (mb - 1) * 128)
                h_new = step(3)
                h_new = step(4)
                blend(2)
                for mb in (4, 5, 6):
                    e_pre(mb, glo[2], (mb - 4) * 128)
                h_new = step(5)
                h_new = step(6)
                blend(3)
                for mb in (7, 8):
                    e_pre(mb, glo[3], (mb - 7) * 128)
                e_pre(EMB_BLOCKS - 1, glo[3], 256, half="lo")
                h_new = step(7)
                h_new = step(8)

            # ============ steady recurrence ============
            for t in range(9, T):
                h_new = step(t)

            h_trans(T - 1, h_new)
            while vstate["q"] < len(vqueue):
                vocab_item()

    nc.finalize()
    return nc


def kernel(**inputs):
    in_maps, meta = _host_prep(inputs)
    nc = build_program(meta)
    res = run_bass_kernel_spmd(nc, in_maps, core_ids=list(range(NCORES)))
    results = res.results

    b_t = meta["b_t"]
    off = meta["off"]
    full = np.zeros((B, T, VPAD), np.float32)
    for k in range(NCORES):
        o = np.asarray(results[k]["out"])
        for t in range(T):
            bt = b_t[t]
            if bt:
                full[:bt, t, k * VS : (k + 1) * VS] = o[off[t] : off[t] + bt]
    return full[:, :, :V]


# revision 25
# speedup vs baseline: 1.3227x; 1.0108x over previous
"""Trainium2 Bass kernel for an attention-LSTM caption decoder.

Math notes (verified against the reference on CPU):
  - num_pixels == 1 makes the softmax attention exactly a no-op: alpha == 1.0,
    so awe = sigmoid(h @ W_beta) * features. W_enc/W_dec/W_full are unused.
  - Masked (b, t) rows (t >= len[b]) never re-activate and never influence
    active rows, so h/c freezing can be dropped; only output masking matters.
    We compute/emit only the active rows (lengths are sorted descending, so
    the active rows at step t are a prefix of the batch).
  - All biases in setup_inputs() are zero; detected host-side, bias work is
    compiled out (fallback paths are kept for nonzero biases).

Distribution (8 cores): recurrence replicated on every core; fc weight and
the output vocab dim sharded 8-way (tensor parallel). The embedding table
(bf16) is resident in each core's HBM.

Schedule highlights vs the v0 kernel:
  - The embedding gather uses dma_gather(transpose=True) on a bf16 table,
    landing rows directly in the transposed [128, kc, row] layout; a zero
    row appended to each table half turns the lo/hi split into a plain add
    (no select mask). 4 swdge queues, gathers issued before the big weight
    DMAs so they overlap.
  - E = emb @ W_ih_emb.T is kept in SBUF (no DRAM staging) and its blocks
    are computed interleaved with the first recurrence steps.
  - The LSTM gate PSUM is split into four independent 512-wide tiles so the
    pointwise chain on chunk f starts while chunks i/o/g are still being
    accumulated on the PE (awe matmuls are emitted f,i,o,g).
  - Vocab-projection matmuls fill the PE idle window during the pointwise
    chain; their PSUM->SBUF copies alternate between ScalarE and VectorE.
"""

import numpy as np

from concourse import bacc, bass, library_config, mybir, tile
from concourse.bass_utils import run_bass_kernel_spmd

F32 = mybir.dt.float32
BF16 = mybir.dt.bfloat16
I16 = mybir.dt.int16

B = 64
H = 512
T = 20
V = 50257
NCORES = 8
VS = 6284            # per-core vocab shard (8 * 6284 = 50272 >= 50257)
VPAD = NCORES * VS
KC_H = H // 128      # 4 k-chunks per 512-wide contraction
GATE_N = 4 * H       # 2048
SPLIT = 32000        # embedding table split (int16 index range)
NHI = V - SPLIT      # 18257 hi rows
Z_HI = NHI           # local index of the zero row in the hi half

N_GATHER = (T - 1) * B           # 1216 gathered rows, j = (t-1)*64 + b
EMB_BLOCKS = 10                  # E blocks of 128 rows (1280 incl features)
FEAT_OFF = N_GATHER              # features rows live at 1216..1279 (block 9 hi)
# 128-aligned groups: g0 small so steps 1-2 unblock as early as possible
GATHER_GROUPS = [(0, 128), (128, 512), (512, 896), (896, 1280)]
VCHUNKS = [(i * 512, min(512, VS - i * 512)) for i in range(13)]
VOCAB_CAP = 5                    # max vocab items interleaved per step


def _pack_k(w):
    """[K, N] -> [128, K//128, N] with the contraction dim on partitions."""
    k, n = w.shape
    assert k % 128 == 0
    return np.ascontiguousarray(w.reshape(k // 128, 128, n).transpose(1, 0, 2))


def _pack_idx(a):
    """(n,) int16 -> [128, n//16]; j = s*16 + p wrapping, replicated into each
    of the 8 GPSIMD Q7 cores' 16-partition groups."""
    n = a.shape[0]
    assert n % 16 == 0
    out = np.zeros((128, n // 16), np.int16)
    for c in range(8):
        out[16 * c : 16 * c + 16, :] = a.reshape(n // 16, 16).T
    return out


def _host_prep(inputs):
    import ml_dtypes

    bf16 = ml_dtypes.bfloat16
    f32 = np.float32
    feats = np.asarray(inputs["features"], f32)
    caps = np.asarray(inputs["captions"]).astype(np.int64)
    lens = np.asarray(inputs["lengths"]).reshape(-1).astype(np.int64)
    table = np.asarray(inputs["embed_table"], f32)

    W_ih = np.asarray(inputs["W_ih"], f32)
    W_hh = np.asarray(inputs["W_hh"], f32)
    b_ih = np.asarray(inputs["b_ih"], f32)
    b_hh = np.asarray(inputs["b_hh"], f32)
    b_beta = np.asarray(inputs["b_beta"], f32)
    b_fc = np.asarray(inputs["b_fc"], f32)
    b_hinit = np.asarray(inputs["b_hinit"], f32)
    b_cinit = np.asarray(inputs["b_cinit"], f32)

    # ragged-batch packing (lengths sorted descending by construction)
    b_t = [int((lens > t).sum()) for t in range(T)]
    off = np.concatenate([[0], np.cumsum(b_t)]).astype(np.int64)
    p_total = int(off[-1])
    p_pad = ((p_total + 127) // 128) * 128

    # bf16 table with zero rows closing the lo/hi split:
    #   rows 0..SPLIT-1   = table[:SPLIT],  row SPLIT = 0   (lo zero)
    #   rows SPLIT+1..    = table[SPLIT:],  last row  = 0   (hi zero)
    tableg = np.zeros((V + 2, H), bf16)
    tableg[:SPLIT] = table[:SPLIT].astype(bf16)
    tableg[SPLIT + 1 : V + 1] = table[SPLIT:].astype(bf16)

    # gather indices, t-major (t=1..19); 64 pad rows -> zero rows
    idx_flat = np.full(EMB_BLOCKS * 128, SPLIT, np.int64)
    idx_flat[:N_GATHER] = caps.T.reshape(-1)
    is_hi = idx_flat >= SPLIT
    idx_lo = np.where(is_hi, SPLIT, idx_flat).astype(np.int16)
    idx_hi = np.where(is_hi, idx_flat - SPLIT, Z_HI).astype(np.int16)

    w2emb = W_ih.T[:H]                      # [512, 2048] emb input rows
    w2ah = np.vstack([W_ih.T[H:], W_hh.T])  # [1024, 2048] awe+h input rows

    b2 = b_ih + b_hh
    has_b2 = bool(np.any(b2))
    has_bbeta = bool(np.any(b_beta))
    has_bfc = bool(np.any(b_fc))
    has_binit = bool(np.any(b_hinit)) or bool(np.any(b_cinit))

    common = {
        "tableg": tableg,
        "featT": _pack_k(feats.T.astype(f32)).astype(bf16),
        "featsb": feats.astype(bf16),
        "w2e": _pack_k(w2emb).astype(bf16),
        "w2ah": _pack_k(w2ah).astype(bf16),
        "wbeta": _pack_k(np.asarray(inputs["W_beta"], f32)).astype(bf16),
        "whinit": _pack_k(np.asarray(inputs["W_hinit"], f32)).astype(bf16),
        "wcinit": _pack_k(np.asarray(inputs["W_cinit"], f32)).astype(bf16),
        "ident": np.eye(128, dtype=f32).astype(bf16),
    }
    idh = np.zeros((128, 64), f32)
    for i in range(64):
        idh[64 + i, i] = 1.0
    common["identhi"] = idh.astype(bf16)
    for g, (a, b) in enumerate(GATHER_GROUPS):
        common[f"idxlo{g}"] = _pack_idx(idx_lo[a:b])
        common[f"idxhi{g}"] = _pack_idx(idx_hi[a:b])
    if has_b2:
        common["b2rep"] = np.ascontiguousarray(
            np.tile(b2[None, :], (128, 1)).astype(f32)
        )
    if has_bbeta or has_binit:
        common["ones"] = np.ones((1, B), bf16)
    if has_bbeta:
        common["bbetarow"] = b_beta.reshape(1, H).astype(bf16)
    if has_binit:
        common["bhinitT"] = np.ascontiguousarray(
            b_hinit.reshape(KC_H, 128).T.astype(f32)
        )
        common["bcinitrow"] = b_cinit.reshape(1, H).astype(bf16)

    W_fc = np.asarray(inputs["W_fc"], f32)
    wfc_pad = np.zeros((H, VPAD), f32)
    wfc_pad[:, :V] = W_fc
    bfc_pad = np.zeros(VPAD, f32)
    bfc_pad[:V] = b_fc

    in_maps = []
    for k in range(NCORES):
        m = dict(common)
        m["wfc"] = _pack_k(wfc_pad[:, k * VS : (k + 1) * VS]).astype(bf16)
        if has_bfc:
            m["bfcrep"] = np.ascontiguousarray(
                np.tile(bfc_pad[k * VS : (k + 1) * VS][None, :], (128, 1))
            ).astype(f32)
        in_maps.append(m)

    meta = {
        "b_t": b_t, "off": off, "p_total": p_total, "p_pad": p_pad,
        "has_b2": has_b2, "has_bbeta": has_bbeta, "has_bfc": has_bfc,
        "has_binit": has_binit,
    }
    return in_maps, meta


def build_program(meta):
    """Build the (SPMD-identical) Bass program. Per-core differences are data
    only (wfc shards)."""
    b_t = meta["b_t"]
    off = [int(x) for x in meta["off"]]
    p_total = meta["p_total"]
    p_pad = meta["p_pad"]
    mv = p_pad // 128
    has_b2 = meta["has_b2"]
    has_bbeta = meta["has_bbeta"]
    has_bfc = meta["has_bfc"]
    has_binit = meta["has_binit"]

    nc = bacc.Bacc(num_swdge_queues=4)

    tableg_d = nc.declare_dram_parameter("tableg", [V + 2, H], BF16, isOutput=False)
    featT_d = nc.declare_dram_parameter("featT", [128, KC_H, B], BF16, isOutput=False)
    featsb_d = nc.declare_dram_parameter("featsb", [B, H], BF16, isOutput=False)
    w2e_d = nc.declare_dram_parameter("w2e", [128, KC_H, GATE_N], BF16, isOutput=False)
    w2ah_d = nc.declare_dram_parameter("w2ah", [128, 8, GATE_N], BF16, isOutput=False)
    wbeta_d = nc.declare_dram_parameter("wbeta", [128, KC_H, H], BF16, isOutput=False)
    whinit_d = nc.declare_dram_parameter("whinit", [128, KC_H, H], BF16, isOutput=False)
    wcinit_d = nc.declare_dram_parameter("wcinit", [128, KC_H, H], BF16, isOutput=False)
    ident_d = nc.declare_dram_parameter("ident", [128, 128], BF16, isOutput=False)
    identhi_d = nc.declare_dram_parameter("identhi", [128, 64], BF16, isOutput=False)
    idx_d = {}
    for g, (a, b) in enumerate(GATHER_GROUPS):
        w = (b - a) // 16
        idx_d[("lo", g)] = nc.declare_dram_parameter(f"idxlo{g}", [128, w], I16, isOutput=False)
        idx_d[("hi", g)] = nc.declare_dram_parameter(f"idxhi{g}", [128, w], I16, isOutput=False)
    wfc_d = nc.declare_dram_parameter("wfc", [128, KC_H, VS], BF16, isOutput=False)
    if has_b2:
        b2rep_d = nc.declare_dram_parameter("b2rep", [128, GATE_N], F32, isOutput=False)
    if has_bbeta or has_binit:
        ones_d = nc.declare_dram_parameter("ones", [1, B], BF16, isOutput=False)
    if has_bbeta:
        bbetarow_d = nc.declare_dram_parameter("bbetarow", [1, H], BF16, isOutput=False)
    if has_binit:
        bhinitT_d = nc.declare_dram_parameter("bhinitT", [128, KC_H], F32, isOutput=False)
        bcinitrow_d = nc.declare_dram_parameter("bcinitrow", [1, H], BF16, isOutput=False)
    if has_bfc:
        bfcrep_d = nc.declare_dram_parameter("bfcrep", [128, VS], F32, isOutput=False)
    out_d = nc.declare_dram_parameter("out", [p_pad, VS], F32, isOutput=True)

    def mm(out, lhsT, rhs, start, stop):
        nc.tensor.matmul(out, lhsT, rhs, start=start, stop=stop)

    # vocab work item (mc, vo, vw) is runnable once all hall rows of block mc
    # are written, i.e. after step mc_ready[mc]'s h transposes.
    mc_ready = []
    for mc in range(mv):
        need = (mc + 1) * 128
        r = T - 1
        for t in range(T):
            if off[t + 1] >= need:
                r = t
                break
        mc_ready.append(r)
    vqueue = [(mc, vo, vw) for mc in range(mv) for (vo, vw) in VCHUNKS]

    # per-step hall write segments: (mc, local_lo, src_lo, n)
    hall_segs = []
    for t in range(T):
        segs = []
        lo, n = off[t], b_t[t]
        while n > 0:
            mc = lo // 128
            ll = lo % 128
            take = min(128 - ll, n)
            segs.append((mc, ll, lo - off[t], take))
            lo += take
            n -= take
        hall_segs.append(segs)

    SIG = mybir.ActivationFunctionType.Sigmoid
    TANH = mybir.ActivationFunctionType.Tanh

    with tile.TileContext(nc) as tc:
        nc.gpsimd.load_library(library_config.mlp)
        with (
            tc.tile_pool(name="const", bufs=1) as constp,
            tc.tile_pool(name="res", bufs=1) as resp,
            tc.tile_pool(name="state", bufs=1) as statep,
            tc.tile_pool(name="step", bufs=1) as sp,
            tc.tile_pool(name="stage", bufs=3) as stp,
            tc.tile_pool(name="gpsum", bufs=1, space="PSUM") as gps,
            tc.tile_pool(name="fill", bufs=2, space="PSUM") as fillp,
            tc.tile_pool(name="tpsum", bufs=2, space="PSUM") as tps,
        ):
            # --- tiny constants first so their DMAs head the queues ---
            ident = constp.tile([128, 128], BF16)
            nc.sync.dma_start(ident[:], ident_d[:])
            identhi = constp.tile([128, 64], BF16)
            nc.sync.dma_start(identhi[:], identhi_d[:])
            featT = constp.tile([128, KC_H, B], BF16)
            nc.sync.dma_start(featT[:], featT_d[:])
            featsb = constp.tile([B, H], BF16)
            nc.sync.dma_start(featsb[:], featsb_d[:])
            if has_bbeta or has_binit:
                ones = constp.tile([1, B], BF16)
                nc.sync.dma_start(ones[:], ones_d[:])
            if has_bbeta:
                bbetarow = constp.tile([1, H], BF16)
                nc.sync.dma_start(bbetarow[:], bbetarow_d[:])

            # --- persistent state (DMAs started after the gathers) ---
            w2ah = resp.tile([128, 8, GATE_N], BF16)
            wbeta = resp.tile([128, KC_H, H], BF16)
            e_sb = resp.tile([128, EMB_BLOCKS, GATE_N], BF16, name="E")
            hall_t = [
                resp.tile([128, KC_H, 128], BF16, tag=f"hall{mc}", name=f"hall{mc}")
                for mc in range(mv)
            ]
            if p_pad > p_total:
                mc = p_total // 128
                nc.vector.memset(hall_t[mc][:, :, p_total % 128 :], 0.0)
                for m2 in range(mc + 1, mv):
                    nc.vector.memset(hall_t[m2][:], 0.0)
            if has_b2:
                b2rep = resp.tile([128, GATE_N], F32)
            if has_bfc:
                bfcrep = resp.tile([128, VS], F32)

            wfc = resp.tile([128, KC_H, VS], BF16)

            hT = statep.tile([128, KC_H, B], BF16)
            aweT = statep.tile([128, KC_H, B], BF16)
            c_st = statep.tile([B, H], F32)

            # gate PSUM: four independent 512-wide chunks (i, f, g, o)
            g_ps = [gps.tile([B, H], F32, tag=f"g{x}", name=f"g{x}")
                    for x in range(4)]

            # =============== helpers ===============
            vstate = {"q": 0, "alt": 0}

            def vocab_item():
                mc, vo, vw = vqueue[vstate["q"]]
                vstate["q"] += 1
                pv = fillp.tile([128, H], F32, tag="fill")
                for kc in range(KC_H):
                    mm(pv[:, :vw], hall_t[mc][:, kc, :], wfc[:, kc, vo : vo + vw],
                       start=(kc == 0), stop=(kc == KC_H - 1))
                st = stp.tile([128, H], F32, tag="st")
                if has_bfc:
                    nc.vector.tensor_add(st[:, :vw], pv[:, :vw],
                                         bfcrep[:, vo : vo + vw])
                elif vstate["alt"] == 0:
                    nc.scalar.copy(st[:, :vw], pv[:, :vw])
                else:
                    nc.vector.tensor_copy(st[:, :vw], pv[:, :vw])
                vstate["alt"] ^= 1
                nc.sync.dma_start(
                    out_d[mc * 128 : (mc + 1) * 128, vo : vo + vw], st[:, :vw]
                )

            def emit_vocab(t):
                n = 0
                while (vstate["q"] < len(vqueue) and n < VOCAB_CAP
                       and mc_ready[vqueue[vstate["q"]][0]] < t):
                    vocab_item()
                    n += 1

            ep_alt = [0]

            def e_pre(mb, lhsT_tile, lhsT_col0, half=None):
                """E block mb from transposed emb rows [128, KC_H, *] at
                lhsT_tile[:, kc, lhsT_col0:+w]. half: None=128 rows,
                'lo'/'hi' = 64-row half blocks."""
                r0, rn = (0, 128) if half is None else ((0, 64) if half == "lo" else (64, 128))
                w = rn - r0
                for nc4 in range(4):
                    pse = fillp.tile([128, H], F32, tag="fill")
                    for kc in range(KC_H):
                        mm(pse[r0:rn, :],
                           lhsT_tile[:, kc, lhsT_col0 + r0 : lhsT_col0 + r0 + w],
                           w2e[:, kc, nc4 * 512 : (nc4 + 1) * 512],
                           start=(kc == 0), stop=(kc == KC_H - 1))
                    dst = e_sb[r0:rn, mb, nc4 * 512 : (nc4 + 1) * 512]
                    if has_b2:
                        nc.vector.tensor_add(
                            dst, pse[r0:rn, :],
                            b2rep[r0:rn, nc4 * 512 : (nc4 + 1) * 512])
                    elif ep_alt[0] == 0:
                        nc.scalar.copy(dst, pse[r0:rn, :])
                    else:
                        nc.vector.tensor_copy(dst, pse[r0:rn, :])
                    ep_alt[0] ^= 1

            def e_load(t):
                """Start the gate accumulation groups with E_t (+ implicitly
                the biases, folded into E)."""
                m0 = FEAT_OFF if t == 0 else (t - 1) * B
                p0, gslc = m0 % 128, m0 // 128
                idx = ident[0:64, 0:64] if p0 == 0 else identhi[64:128, 0:64]
                for x in range(4):
                    mm(g_ps[x][:], idx, e_sb[p0 : p0 + B, gslc, x * 512 : (x + 1) * 512],
                       start=True, stop=False)

            def h_trans(t, h_new):
                """Transpose h_new (= h_{t+1}, preds source of step t) into hT
                and scatter its active prefix into the hall tiles."""
                for kc in range(KC_H):
                    tp = tps.tile([128, B], BF16, tag="tp")
                    nc.tensor.transpose(
                        tp[:, 0:B], h_new[:, kc * 128 : (kc + 1) * 128],
                        ident[0:B, 0:B])
                    nc.vector.tensor_copy(hT[:, kc, :], tp[:])
                    for (mc, ll, sl, seg_n) in hall_segs[t]:
                        nc.vector.tensor_copy(
                            hall_t[mc][:, kc, ll : ll + seg_n],
                            tp[:, sl : sl + seg_n])

            def beta_mms():
                """beta = h @ W_beta [+ b_beta] into PSUM; returns the tile."""
                betaps = fillp.tile([128, H], F32, tag="fill")
                for kc in range(KC_H):
                    mm(betaps[0:B, :], hT[:, kc, :], wbeta[:, kc, :],
                       start=(kc == 0), stop=(kc == KC_H - 1) and not has_bbeta)
                if has_bbeta:
                    mm(betaps[0:B, :], ones[:], bbetarow[:], start=False, stop=True)
                return betaps

            def h_part():
                """h contribution to the gates (E already loaded)."""
                for kc in range(KC_H):
                    for x in range(4):
                        mm(g_ps[x][:], hT[:, kc, :],
                           w2ah[:, 4 + kc, x * 512 : (x + 1) * 512],
                           start=False, stop=False)

            def sig_awe(betaps):
                """awe = sigmoid(beta) * features, transposed into aweT. The
                ACT/DVE work overlaps the h_part matmuls on the PE."""
                sigb = sp.tile([B, H], BF16, tag="sigb")
                nc.scalar.activation(sigb[:], betaps[0:B, :], SIG)
                aweb = sp.tile([B, H], BF16, tag="aweb")
                nc.vector.tensor_mul(aweb[:], sigb[:], featsb[:])
                for kc in range(KC_H):
                    tp = tps.tile([128, B], BF16, tag="tp")
                    nc.tensor.transpose(
                        tp[:, 0:B], aweb[:, kc * 128 : (kc + 1) * 128],
                        ident[0:B, 0:B])
                    nc.vector.tensor_copy(aweT[:, kc, :], tp[:])

            def awe_and_pointwise(t):
                """awe gate matmuls + LSTM pointwise; returns h_new tile."""
                # awe contribution, chunk-major f,i,o,g so f completes first
                for x in (1, 0, 3, 2):
                    for kc in range(KC_H):
                        mm(g_ps[x][:], aweT[:, kc, :],
                           w2ah[:, kc, x * 512 : (x + 1) * 512],
                           start=False, stop=(kc == KC_H - 1))
                # pointwise (PyTorch gate order i, f, g, o)
                sig_f = sp.tile([B, H], F32, tag="sig_f")
                nc.scalar.activation(sig_f[:], g_ps[1][:], SIG)
                sig_i = sp.tile([B, H], F32, tag="sig_i")
                nc.scalar.activation(sig_i[:], g_ps[0][:], SIG)
                sig_o = sp.tile([B, H], BF16, tag="sig_o")
                nc.scalar.activation(sig_o[:], g_ps[3][:], SIG)
                tanh_g = sp.tile([B, H], F32, tag="tanh_g")
                nc.scalar.activation(tanh_g[:], g_ps[2][:], TANH)
                nc.vector.tensor_mul(c_st[:], c_st[:], sig_f[:])
                t2 = sp.tile([B, H], F32, tag="t2")
                nc.vector.tensor_mul(t2[:], sig_i[:], tanh_g[:])
                nc.vector.tensor_add(c_st[:], c_st[:], t2[:])
                tanh_c = sp.tile([B, H], BF16, tag="tanh_c")
                nc.scalar.activation(tanh_c[:], c_st[:], TANH)
                h_new = sp.tile([B, H], BF16, tag="h_new")
                nc.vector.tensor_mul(h_new[:], sig_o[:], tanh_c[:])
                return h_new

            # ================= prep phase =================
            with (
                tc.tile_pool(name="prew", bufs=1) as prew,
                tc.tile_pool(name="gath", bufs=2) as gp,
            ):
                glo = [None] * 4
                ghi = [None] * 4

                def gather(g):
                    a, b = GATHER_GROUPS[g]
                    n = b - a
                    w = n // 16
                    idxlo = gp.tile([128, w], I16, tag=f"ilo{g}", bufs=1,
                                    name=f"ilo{g}")
                    idxhi = gp.tile([128, w], I16, tag=f"ihi{g}", bufs=1,
                                    name=f"ihi{g}")
                    nc.sync.dma_start(idxlo[:], idx_d[("lo", g)][:])
                    nc.sync.dma_start(idxhi[:], idx_d[("hi", g)][:])
                    glo[g] = gp.tile([128, KC_H, n], BF16, tag=f"glo{g}",
                                     bufs=1, name=f"glo{g}")
                    ghi[g] = gp.tile([128, KC_H, n], BF16, tag=f"ghi{g}",
                                     bufs=1, name=f"ghi{g}")
                    nc.gpsimd.dma_gather(
                        glo[g][:], tableg_d[0 : SPLIT + 1, :],
                        idxlo[:], num_idxs=n, num_idxs_reg=n,
                        elem_size=H, transpose=True, queue_num=0,
                    )
                    nc.gpsimd.dma_gather(
                        ghi[g][:], tableg_d[SPLIT + 1 : V + 2, :],
                        idxhi[:], num_idxs=n, num_idxs_reg=n,
                        elem_size=H, transpose=True, queue_num=0,
                    )

                def blend(g):
                    a, b = GATHER_GROUPS[g]
                    n = b - a
                    nc.vector.tensor_add(glo[g][:], glo[g][:], ghi[g][:])

                # gathers first: their DMAs race ahead of the weight loads
                for g in range(4):
                    gather(g)

                # warm the sigmoid/tanh ACT table during the DMA wait
                warm = sp.tile([1, 2], F32, tag="warm")
                nc.scalar.activation(warm[:], ident[0:1, 0:2], SIG)

                # weight DMAs, ordered by first use
                nc.scalar.dma_start(wbeta[:], wbeta_d[:])
                w2e = prew.tile([128, KC_H, GATE_N], BF16, bufs=1)
                nc.scalar.dma_start(w2e[:], w2e_d[:])
                nc.scalar.dma_start(w2ah[:], w2ah_d[:])
                if has_b2:
                    nc.scalar.dma_start(b2rep[:], b2rep_d[:])
                if has_bfc:
                    nc.scalar.dma_start(bfcrep[:], bfcrep_d[:])

                # h0 (transposed directly) and c0 while gathers fly
                with tc.tile_pool(name="initp", bufs=1) as ip:
                    whinit = ip.tile([128, KC_H, H], BF16)
                    nc.sync.dma_start(whinit[:], whinit_d[:])
                    wcinit = ip.tile([128, KC_H, H], BF16)
                    nc.sync.dma_start(wcinit[:], wcinit_d[:])
                    if has_binit:
                        bhinitT = ip.tile([128, KC_H], F32)
                        nc.sync.dma_start(bhinitT[:], bhinitT_d[:])
                        bcinitrow = ip.tile([1, H], BF16)
                        nc.sync.dma_start(bcinitrow[:], bcinitrow_d[:])
                    for jb in range(KC_H):
                        hps = fillp.tile([128, H], F32, tag="fill")
                        for kc in range(KC_H):
                            mm(hps[:, 0:B], whinit[:, kc, jb * 128 : (jb + 1) * 128],
                               featT[:, kc, :], start=(kc == 0), stop=(kc == KC_H - 1))
                        if has_binit:
                            nc.scalar.activation(
                                hT[:, jb, :], hps[:, 0:B],
                                mybir.ActivationFunctionType.Identity,
                                bias=bhinitT[:, jb : jb + 1])
                        else:
                            nc.vector.tensor_copy(hT[:, jb, :], hps[:, 0:B])
                    cps = fillp.tile([128, H], F32, tag="fill")
                    for kc in range(KC_H):
                        mm(cps[0:B, :], featT[:, kc, :], wcinit[:, kc, :],
                           start=(kc == 0), stop=(kc == KC_H - 1) and not has_binit)
                    if has_binit:
                        mm(cps[0:B, :], ones[:], bcinitrow[:], start=False, stop=True)
                    nc.vector.tensor_copy(c_st[:], cps[0:B, :])

                # E for t=0 (features block = hi half of block 9; featT holds
                # exactly those 64 rows, hence the -64 column bias)
                e_pre(EMB_BLOCKS - 1, featT, -64, half="hi")

                # wfc DMA after the early weights (first vocab item is t>=3)
                nc.scalar.dma_start(wfc[:], wfc_d[:])

                # ---- step 0 ----
                e_load(0)
                bps = beta_mms()
                h_part()
                sig_awe(bps)
                h_new = awe_and_pointwise(0)

                # ---- steps 1..8 with interleaved gather/E work ----
                def step(t):
                    e_load(t)
                    h_trans(t - 1, h_new)
                    bps = beta_mms()
                    h_part()
                    sig_awe(bps)
                    hn = awe_and_pointwise(t)
                    emit_vocab(t)
                    return hn

                # E-block needs: step t reads block (t-1)//2
                blend(0)
                e_pre(0, glo[0], 0)
                h_new = step(1)
                h_new = step(2)
                blend(1)
                for mb in (1, 2, 3):
                    e_pre(mb, glo[1], (mb - 1) * 128)
                h_new = step(3)
                h_new = step(4)
                blend(2)
                for mb in (4, 5, 6):
                    e_pre(mb, glo[2], (mb - 4) * 128)
                h_new = step(5)
                h_new = step(6)
                blend(3)
                for mb in (7, 8):
                    e_pre(mb, glo[3], (mb - 7) * 128)
                e_pre(EMB_BLOCKS - 1, glo[3], 256, half="lo")
                h_new = step(7)
                h_new = step(8)

            # ============ steady recurrence ============
            for t in range(9, T):
                h_new = step(t)

            h_trans(T - 1, h_new)
            while vstate["q"] < len(vqueue):
                vocab_item()

    nc.finalize()
    return nc


def kernel(**inputs):
    in_maps, meta = _host_prep(inputs)
    nc = build_program(meta)
    res = run_bass_kernel_spmd(nc, in_maps, core_ids=list(range(NCORES)))
    results = res.results

    b_t = meta["b_t"]
    off = meta["off"]
    full = np.zeros((B, T, VPAD), np.float32)
    for k in range(NCORES):
        o = np.asarray(results[k]["out"])
        for t in range(T):
            bt = b_t[t]
            if bt:
                full[:bt, t, k * VS : (k + 1) * VS] = o[off[t] : off[t] + bt]
    return full[:, :, :V]
